# revision 7
# baseline (speedup 1.0000x reference)
"""Trainium2 Bass kernel for nn_NlEncoder (GNN message passing + transformer).

Sharding: N=2048 rows split across 8 cores (256 rows each). Each layer: every
core computes its row-block of scores/attention against all-gathered key-side
tensors, then local GRU/FFN. AllGather of transposed local blocks between
layers. Embedding gather + tiny output head run on host.

Device layouts per core:
  natural    [128, NB, X]: row n = b*128+p on partitions, features free
  transposed [128, KE, N]: feature e = k*128+p on partitions, rows free
f32 matmuls issue as float32r (full-rate PE) via AP bitcast; the attention
inner loop (scores / exp / att@V) runs in bf16 operands with f32 PSUM accum.
"""

import contextlib
import numpy as np
import ml_dtypes

import concourse.bass as bass
import concourse.bacc as bacc
import concourse.tile as tile
from concourse import mybir
from concourse.bass_utils import run_bass_kernel_spmd

R = 8
T = 2048
NL = T // R          # 256
E = 256
H = 8
D = 32
FFD = 1024
NTB = 5
NJB = T // 128       # 16
KE = E // 128        # 2
NB = NL // 128       # 2
ALPHA = 0.2
NLLEN = 1024

f32 = mybir.dt.float32
f32r = mybir.dt.bfloat16  # matmul operand dtype (PE full rate, FWL)
bf16 = mybir.dt.bfloat16
AF = mybir.ActivationFunctionType
ALU = mybir.AluOpType


def r32(ap):
    assert ap.dtype != f32, f"f32 operand reached matmul: {ap}"
    return ap


def build_bass(n_tb=NTB, n_gg=2):
    nc = bacc.Bacc("TRN2", num_devices=R, name="nlenc")
    with tile.TileContext(nc) as tc:
        _build_body(nc, tc, n_tb, n_gg)
    nc.compile()
    return nc


def _build_body(nc, tc, n_tb, n_gg):
    din = {}

    def inp(name, shape, dtype=f32):
        din[name] = nc.dram_tensor(name, list(shape), dtype,
                                   kind="ExternalInput")
        return din[name]

    def dap(name):
        h = din[name]
        return h[tuple(slice(None) for _ in h.shape)]

    inp("h0T_full", [128, KE, T], f32r)
    inp("h0_loc", [128, NB, E])
    inp("h0_locT", [128, KE, NL], f32r)
    inp("adjT", [128, NJB, NL], bf16)
    inp("m_tiles", [128, NJB])
    inp("m32", [128, NJB, 32], bf16)
    inp("ident", [128, 128])
    for g in range(n_gg):
        inp(f"g{g}_Wx", [128, KE, E + 1], f32r)
        inp(f"g{g}_Wa1", [128, KE, 1], f32r)
        inp(f"g{g}_WiT", [128, KE, 3 * E], f32r)
        inp(f"g{g}_WhT", [128, KE, 3 * E], f32r)
        inp(f"g{g}_bi", [1, 3 * E], f32r)
        inp(f"g{g}_bh", [1, 3 * E], f32r)
    TBW = [("wq", [128, KE, E], f32r), ("wk", [128, KE, E], f32r),
           ("wv", [128, KE, E], f32r), ("wo", [128, KE, E], f32r),
           ("bq", [128, KE, 1], f32), ("bk", [128, KE, 1], f32),
           ("bv", [1, E], f32r), ("bo", [1, E], f32r),
           ("w1", [128, KE, FFD], f32r), ("b1", [128, FFD // 128, 1], f32),
           ("w2", [128, FFD // 128, E], bf16), ("b2", [1, E], f32r),
           ("ln1g", [1, E], f32), ("ln1b", [1, E], f32),
           ("ln2g", [1, E], f32), ("ln2b", [1, E], f32)]
    for l in range(n_tb):
        for nm, shp, dt_ in TBW:
            inp(f"t{l}_{nm}", shp, dt_)

    out_x = nc.dram_tensor("out_x", [NB, 128, E], f32, kind="ExternalOutput")

    ctx = contextlib.ExitStack()
    with ctx:
        sing = ctx.enter_context(tc.tile_pool(name="sing", bufs=1))
        state = ctx.enter_context(tc.tile_pool(name="state", bufs=2))
        big = ctx.enter_context(tc.tile_pool(name="bigsb", bufs=1))
        wpool = ctx.enter_context(tc.tile_pool(name="wpool", bufs=1))
        work = ctx.enter_context(tc.tile_pool(name="work", bufs=1))
        spool = ctx.enter_context(tc.tile_pool(name="spool", bufs=2))
        ps1 = ctx.enter_context(tc.tile_pool(name="ps1", bufs=4, space="PSUM"))
        ps2 = ctx.enter_context(tc.tile_pool(name="ps2", bufs=2, space="PSUM"))
        dram = ctx.enter_context(tc.tile_pool(name="dram", bufs=2,
                                              space="DRAM"))

        def mm(out, lhsT, rhs, **kw):
            nc.tensor.matmul(out, r32(lhsT), r32(rhs), **kw)

        # ---- constants ----
        ident = sing.tile([128, 128], f32, name="identc")
        nc.sync.dma_start(out=ident, in_=dap("ident"))
        ones_row = sing.tile([1, 128], f32r, name="ones_row")
        nc.vector.memset(ones_row, 1.0)
        eps_t = sing.tile([128, 1], f32, name="eps_t")
        nc.vector.memset(eps_t, 1e-5)
        adjT = sing.tile([128, NJB, NL], bf16, name="adjTc")
        nc.sync.dma_start(out=adjT, in_=dap("adjT"))
        m_tiles = sing.tile([128, NJB], f32, name="m_tilesc")
        nc.sync.dma_start(out=m_tiles, in_=dap("m_tiles"))
        m32 = sing.tile([128, NJB, 32], bf16, name="m32c")
        nc.sync.dma_start(out=m32, in_=dap("m32"))

        h_loc = state.tile([128, NB, E], f32, name="h_loc", tag="h_loc")
        h_locT = state.tile([128, KE, NL], f32r, name="h_locT", tag="h_locT")
        hT_full = big.tile([128, KE, T], f32r, name="hT_full", tag="hT_full",
                           bufs=2)
        nc.sync.dma_start(out=h_loc, in_=dap("h0_loc"))
        nc.sync.dma_start(out=h_locT, in_=dap("h0_locT"))
        nc.sync.dma_start(out=hT_full, in_=dap("h0T_full"))

        def cp(i, out, in_):
            if i % 2:
                nc.scalar.copy(out=out, in_=in_)
            else:
                nc.vector.tensor_copy(out=out, in_=in_)

        def transpose_to(dst, src_nat):
            for b in range(NB):
                for k in range(KE):
                    tp = ps1.tile([128, 512], f32, name="tp", tag="ps1")
                    nc.tensor.transpose(
                        tp[:, :128], src_nat[:, b, k * 128:(k + 1) * 128],
                        ident)
                    cp(b * KE + k, dst[:, k, b * 128:(b + 1) * 128],
                       tp[:, :128])

        def allgather(srcT, dstT_full):
            cont = dram.tile([KE, 128, NL], f32r, name="agin", tag="agin")
            gout = dram.tile([R, KE, 128, NL], f32r, name="agout", tag="agout",
                             addr_space="Shared")
            nc.sync.dma_start(out=cont.rearrange("k p n -> p k n"), in_=srcT)
            nc.gpsimd.collective_compute(
                "AllGather", ALU.bypass,
                replica_groups=[list(range(R))],
                ins=[cont.opt()], outs=[gout.opt()],
            )
            for k in range(KE):
                for rh in range(2):
                    nc.sync.dma_start(
                        out=dstT_full[:, k, rh * (T // 2):(rh + 1) * (T // 2)]
                            .rearrange("p (r n) -> p r n", r=R // 2),
                        in_=gout[rh * (R // 2):(rh + 1) * (R // 2), k, :, :]
                            .rearrange("r p n -> p r n"),
                    )

        # ================= GGANN =================
        for g in range(n_gg):
            Wx = wpool.tile([128, KE, E + 1], f32r, name=f"Wx{g}", tag="wsmall",
                            bufs=8)
            Wa1 = wpool.tile([128, KE, 1], f32r, name=f"Wa1{g}", tag="wcol",
                             bufs=8)
            WiT = wpool.tile([128, KE, 3 * E], f32r, name=f"WiT{g}", tag="wbig",
                             bufs=3)
            WhT = wpool.tile([128, KE, 3 * E], f32r, name=f"WhT{g}", tag="wbig",
                             bufs=3)
            bi_r = wpool.tile([1, 3 * E], f32r, name=f"bi{g}", tag="wvec",
                              bufs=4)
            bh_r = wpool.tile([1, 3 * E], f32r, name=f"bh{g}", tag="wvec",
                              bufs=4)
            for nm, tl in [("Wx", Wx), ("Wa1", Wa1), ("WiT", WiT),
                           ("WhT", WhT), ("bi", bi_r), ("bh", bh_r)]:
                nc.sync.dma_start(out=tl, in_=dap(f"g{g}_{nm}"))

            # Wh(+e2) per j-block; Whg bf16 with trailing ones column
            Whg = big.tile([128, NJB, E + 1], bf16, name=f"Whg{g}", tag="bigA")
            e2c = work.tile([128, NJB], f32, name=f"e2c{g}", tag="e2c")
            for jb in range(NJB):
                pw = ps1.tile([128, 512], f32, name="pw", tag="ps1")
                for k in range(KE):
                    mm(pw[:, :E + 1], hT_full[:, k, jb * 128:(jb + 1) * 128],
                       Wx[:, k, :], start=(k == 0), stop=(k == KE - 1))
                cp(jb, Whg[:, jb, :E], pw[:, :E])
                nc.vector.tensor_copy(e2c[:, jb:jb + 1], pw[:, E:E + 1])
                nc.vector.memset(Whg[:, jb, E:E + 1], 1.0)

            pe1 = ps1.tile([128, 512], f32, name="pe1", tag="ps1")
            for k in range(KE):
                mm(pe1[:1, :NL], Wa1[:, k, :], h_locT[:, k, :],
                   start=(k == 0), stop=(k == KE - 1))
            e1row = work.tile([1, NL], f32r, name="e1row", tag="e1row")
            nc.vector.tensor_copy(e1row, pe1[:1, :NL])
            pb = ps1.tile([128, 512], f32, name="pb", tag="ps1")
            mm(pb[:, :NL], ones_row, e1row, start=True, stop=True)
            E1b = work.tile([128, NL], f32, name="E1b", tag="E1b")
            nc.vector.tensor_copy(E1b, pb[:, :NL])

            hp_ps = [ps1.tile([128, 512], f32, name=f"hp{b}", tag="ps1")
                     for b in range(NB)]
            for half in range(2):
                strip = spool.tile([128, NJB // 2, NL], bf16,
                                   name=f"str{g}{half}", tag="ggstrip")
                for j in range(NJB // 2):
                    jb = half * (NJB // 2) + j
                    nc.scalar.activation(
                        out=strip[:, j, :], in_=E1b, func=AF.Lrelu,
                        bias=e2c[:, jb:jb + 1], scale=1.0, alpha=ALPHA)
                nc.scalar.activation(out=strip[:, :, :], in_=strip[:, :, :],
                                     func=AF.Exp)
                nc.vector.tensor_mul(
                    strip[:, :, :], strip[:, :, :],
                    adjT[:, half * (NJB // 2):(half + 1) * (NJB // 2), :])
                for j in range(NJB // 2):
                    jb = half * (NJB // 2) + j
                    for b in range(NB):
                        mm(hp_ps[b][:, :E + 1],
                           strip[:, j, b * 128:(b + 1) * 128],
                           Whg[:, jb, :],
                           start=(jb == 0), stop=(jb == NJB - 1),
                           skip_group_check=True)

            h_p = work.tile([128, NB, E], f32, name=f"h_p{g}", tag="h_p")
            for b in range(NB):
                rsum = work.tile([128, 1], f32, name=f"rs{g}{b}", tag="rsum")
                nc.vector.tensor_scalar_add(rsum, hp_ps[b][:, E:E + 1], 1e-30)
                rrec = work.tile([128, 1], f32, name=f"rr{g}{b}", tag="rrec")
                nc.vector.reciprocal(rrec, rsum)
                nc.vector.tensor_scalar_mul(h_p[:, b, :], hp_ps[b][:, :E],
                                            rrec)

            h_pT = work.tile([128, KE, NL], f32r, name=f"h_pT{g}", tag="h_pT")
            transpose_to(h_pT, h_p)
            h_new = state.tile([128, NB, E], f32, name=f"h_new{g}",
                               tag="h_loc")
            for b in range(NB):
                # r/z gates: gi+gh accumulated jointly in one PSUM tile
                prz = ps1.tile([128, 512], f32, name="prz", tag="ps1")
                mm(prz, ones_row, bi_r[:, 0:512], start=True, stop=False,
                   skip_group_check=True)
                mm(prz, ones_row, bh_r[:, 0:512], start=False, stop=False,
                   skip_group_check=True)
                for k in range(KE):
                    mm(prz, h_pT[:, k, b * 128:(b + 1) * 128],
                       WiT[:, k, 0:512], start=False, stop=False,
                       skip_group_check=True)
                    mm(prz, h_locT[:, k, b * 128:(b + 1) * 128],
                       WhT[:, k, 0:512], start=False, stop=(k == KE - 1),
                       skip_group_check=True)
                # n gate: inn and hn separate
                pin = ps1.tile([128, 512], f32, name="pin", tag="ps1")
                phn = ps1.tile([128, 512], f32, name="phn", tag="ps1")
                mm(pin[:, :E], ones_row, bi_r[:, 512:768], start=True,
                   stop=False, skip_group_check=True)
                mm(phn[:, :E], ones_row, bh_r[:, 512:768], start=True,
                   stop=False, skip_group_check=True)
                for k in range(KE):
                    mm(pin[:, :E], h_pT[:, k, b * 128:(b + 1) * 128],
                       WiT[:, k, 512:768], start=False, stop=(k == KE - 1),
                       skip_group_check=True)
                    mm(phn[:, :E], h_locT[:, k, b * 128:(b + 1) * 128],
                       WhT[:, k, 512:768], start=False, stop=(k == KE - 1),
                       skip_group_check=True)
                tr = work.tile([128, E], f32, name="tr", tag="gr_r")
                tz = work.tile([128, E], f32, name="tz", tag="gr_z")
                tn = work.tile([128, E], f32, name="tn", tag="gr_n")
                t2 = work.tile([128, E], f32, name="t2", tag="gr_t")
                nc.scalar.activation(out=tr, in_=prz[:, 0:E], func=AF.Sigmoid)
                nc.scalar.activation(out=tz, in_=prz[:, E:2 * E],
                                     func=AF.Sigmoid)
                nc.vector.tensor_mul(tn, tr, phn[:, :E])
                nc.vector.tensor_add(tn, tn, pin[:, :E])
                nc.scalar.activation(out=tn, in_=tn, func=AF.Tanh)
                nc.vector.tensor_sub(t2, h_loc[:, b, :], tn)
                nc.vector.tensor_mul(t2, t2, tz)
                nc.vector.tensor_add(h_new[:, b, :], t2, tn)

            if g == 0:
                ex = work.tile([128, NB, E], f32, name="elu_e", tag="elu_e")
                mk = work.tile([128, NB, E], f32, name="elu_m", tag="elu_m")
                nc.scalar.activation(out=ex[:, :, :], in_=h_new[:, :, :],
                                     func=AF.Exp)
                nc.vector.tensor_scalar(mk[:, :, :], h_new[:, :, :], 0.0,
                                        None, op0=ALU.is_gt)
                nc.vector.tensor_scalar_add(ex[:, :, :], ex[:, :, :], -1.0)
                nc.vector.tensor_sub(h_new[:, :, :], h_new[:, :, :],
                                     ex[:, :, :])
                nc.vector.tensor_mul(h_new[:, :, :], h_new[:, :, :],
                                     mk[:, :, :])
                nc.vector.tensor_add(h_new[:, :, :], h_new[:, :, :],
                                     ex[:, :, :])

            h_loc = h_new
            h_locT = state.tile([128, KE, NL], f32r, name=f"h_nT{g}",
                                tag="h_locT")
            transpose_to(h_locT, h_loc)
            if g < n_gg - 1 or n_tb > 0:
                hT_full = big.tile([128, KE, T], f32r, name=f"hTf{g}",
                                   tag="hT_full", bufs=2)
                allgather(h_locT, hT_full)

        # ================= transformer =================
        x_loc, x_locT, xT_full = h_loc, h_locT, hT_full
        for l in range(n_tb):
            w = {}
            for nm, shp, dt_ in TBW:
                if nm in ("wq", "wk", "wv", "wo"):
                    tg, bf = "wsmall", 8
                elif nm in ("w1", "w2"):
                    tg, bf = "wbig", 3
                elif nm in ("bv", "bo", "b2"):
                    tg, bf = "wvec", 4
                elif nm in ("bq", "bk", "b1"):
                    tg, bf = "wcol", 8
                else:
                    tg, bf = None, None
                if tg is not None:
                    w[nm] = wpool.tile(shp, dt_, name=f"{nm}{l}", tag=tg,
                                       bufs=bf)
                    nc.sync.dma_start(out=w[nm], in_=dap(f"t{l}_{nm}"))
            lnb = {}
            for nm in ["ln1g", "ln1b", "ln2g", "ln2b"]:
                t = wpool.tile([128, E], f32, name=f"{nm}b{l}", tag="lnb",
                               bufs=8)
                src = dap(f"t{l}_{nm}")
                nc.sync.dma_start(
                    out=t,
                    in_=bass.AP(tensor=src.tensor, offset=src.offset,
                                ap=[[0, 128], src.ap[-1]]))
                lnb[nm] = t

            QT = work.tile([128, KE, NL], bf16, name=f"QT{l}", tag="QT")
            for t in range(KE):
                pq = ps1.tile([128, 512], f32, name="pq", tag="ps1")
                for k in range(KE):
                    mm(pq[:, :NL], w["wq"][:, k, t * 128:(t + 1) * 128],
                       x_locT[:, k, :], start=(k == 0), stop=(k == KE - 1))
                nc.vector.tensor_scalar(
                    QT[:, t, :], pq[:, :NL], w["bq"][:, t, :],
                    float(1.0 / np.sqrt(D)), op0=ALU.add, op1=ALU.mult)

            KT = big.tile([128, KE, T], bf16, name=f"KT{l}", tag="bigA")
            for t in range(KE):
                for jc in range(T // 512):
                    pk = ps1.tile([128, 512], f32, name="pk", tag="ps1")
                    for k in range(KE):
                        mm(pk, w["wk"][:, k, t * 128:(t + 1) * 128],
                           xT_full[:, k, jc * 512:(jc + 1) * 512],
                           start=(k == 0), stop=(k == KE - 1))
                    nc.vector.tensor_scalar_add(
                        KT[:, t, jc * 512:(jc + 1) * 512], pk,
                        w["bk"][:, t, :])

            Vm = big.tile([128, NJB, E], bf16, name=f"Vm{l}", tag="bigB")
            for jb in range(NJB):
                pv = ps1.tile([128, 512], f32, name="pv", tag="ps1")
                mm(pv[:, :E], ones_row, w["bv"], start=True, stop=False,
                   skip_group_check=True)
                for k in range(KE):
                    mm(pv[:, :E], xT_full[:, k, jb * 128:(jb + 1) * 128],
                       w["wv"][:, k, :], start=False, stop=(k == KE - 1),
                       skip_group_check=True)
                nc.vector.tensor_scalar_mul(Vm[:, jb, :], pv[:, :E],
                                            m_tiles[:, jb:jb + 1])

            OTn = work.tile([128, KE, NL], f32r, name=f"OTn{l}", tag="OTn")
            for q in range(2):
                strips = [spool.tile([128, NJB, NL], bf16,
                                     name=f"u{l}{q}{h4}", tag="ustrip",
                                     bufs=4)
                          for h4 in range(4)]
                for jq in range(4):
                    for hp in range(2):
                        scs = [ps2.tile([128, 1024], f32, name="sc",
                                        tag="ps2") for _ in range(2)]
                        for j in range(4):
                            jb = jq * 4 + j
                            for i in range(2):
                                h4 = hp * 2 + i
                                mm(scs[i][:, j * 256:(j + 1) * 256],
                                   KT[h4 * 32:(h4 + 1) * 32, q,
                                      jb * 128:(jb + 1) * 128],
                                   QT[h4 * 32:(h4 + 1) * 32, q, :],
                                   start=True, stop=True,
                                   tile_position=(h4 * 32, 0),
                                   skip_group_check=True)
                        for i in range(2):
                            h4 = hp * 2 + i
                            nc.scalar.activation(
                                out=strips[h4][:, jq * 4:(jq + 1) * 4, :],
                                in_=scs[i], func=AF.Exp)
                OT_ps = ps1.tile([128, 512], f32, name="otps", tag="ps1")
                RS_ps = ps1.tile([128, 512], f32, name="rsps", tag="ps1")
                for jb in range(NJB):
                    for h4 in range(4):
                        h = q * 4 + h4
                        mm(OT_ps[h4 * 32:(h4 + 1) * 32, :NL],
                           Vm[:, jb, h * 32:(h + 1) * 32],
                           strips[h4][:, jb, :],
                           start=(jb == 0 and h4 == 0), stop=(jb == NJB - 1),
                           tile_position=(0, h4 * 32), skip_group_check=True)
                        mm(RS_ps[h4 * 32:(h4 + 1) * 32, :NL],
                           m32[:, jb, :], strips[h4][:, jb, :],
                           start=(jb == 0 and h4 == 0), stop=(jb == NJB - 1),
                           tile_position=(0, h4 * 32), skip_group_check=True)
                rrec = work.tile([128, NL], f32, name="rrec", tag="rrec")
                nc.vector.reciprocal(rrec, RS_ps[:, :NL])
                nc.vector.tensor_mul(OTn[:, q, :], OT_ps[:, :NL], rrec)

            x2 = work.tile([128, NB, E], f32, name=f"x2{l}", tag="x2")
            for b in range(NB):
                po = ps1.tile([128, 512], f32, name="po", tag="ps1")
                mm(po[:, :E], ones_row, w["bo"], start=True, stop=False,
                   skip_group_check=True)
                for q in range(KE):
                    mm(po[:, :E], OTn[:, q, b * 128:(b + 1) * 128],
                       w["wo"][:, q, :], start=False, stop=(q == KE - 1),
                       skip_group_check=True)
                nc.vector.tensor_add(x2[:, b, :], po[:, :E], x_loc[:, b, :])

            def layernorm(dst, src, gname, bname):
                for b in range(NB):
                    st = work.tile([128, 6], f32, name="st", tag="ln_st")
                    mv = work.tile([128, 2], f32, name="mv", tag="ln_mv")
                    nc.vector.bn_stats(out=st, in_=src[:, b, :])
                    nc.vector.bn_aggr(out=mv, in_=st)
                    sd = work.tile([128, 1], f32, name="sd", tag="ln_sd")
                    nc.scalar.activation(out=sd, in_=mv[:, 1:2], func=AF.Sqrt,
                                         bias=eps_t, scale=1.0)
                    rstd = work.tile([128, 1], f32, name="rstd", tag="ln_rs")
                    nc.vector.reciprocal(rstd, sd)
                    nc.vector.tensor_scalar(dst[:, b, :], src[:, b, :],
                                            mv[:, 0:1], rstd,
                                            op0=ALU.subtract, op1=ALU.mult)
                    nc.vector.tensor_mul(dst[:, b, :], dst[:, b, :],
                                         lnb[gname])
                    nc.vector.tensor_add(dst[:, b, :], dst[:, b, :],
                                         lnb[bname])

            x_ln = work.tile([128, NB, E], f32, name=f"xln{l}", tag="x_ln")
            layernorm(x_ln, x2, "ln1g", "ln1b")

            x_lnT = work.tile([128, KE, NL], f32r, name=f"xlnT{l}",
                              tag="x_lnT")
            transpose_to(x_lnT, x_ln)
            f1 = big.tile([128, FFD // 128, NL], bf16, name=f"f1{l}",
                          tag="bigB")
            for mb in range(FFD // 128):
                pf = ps1.tile([128, 512], f32, name="pf", tag="ps1")
                for k in range(KE):
                    mm(pf[:, :NL], w["w1"][:, k, mb * 128:(mb + 1) * 128],
                       x_lnT[:, k, :], start=(k == 0), stop=(k == KE - 1))
                nc.scalar.activation(out=f1[:, mb, :], in_=pf[:, :NL],
                                     func=AF.Gelu_apprx_tanh,
                                     bias=w["b1"][:, mb, :], scale=1.0)
            x3 = work.tile([128, NB, E], f32, name=f"x3{l}", tag="x3")
            for b in range(NB):
                pf2 = ps1.tile([128, 512], f32, name="pf2", tag="ps1")
                mm(pf2[:, :E], ones_row, w["b2"], start=True, stop=False,
                   skip_group_check=True)
                for km in range(FFD // 128):
                    mm(pf2[:, :E], f1[:, km, b * 128:(b + 1) * 128],
                       w["w2"][:, km, :], start=False,
                       stop=(km == FFD // 128 - 1), skip_group_check=True)
                nc.vector.tensor_add(x3[:, b, :], pf2[:, :E], x_ln[:, b, :])

            x_new = state.tile([128, NB, E], f32, name=f"xn{l}", tag="h_loc")
            layernorm(x_new, x3, "ln2g", "ln2b")

            x_loc = x_new
            x_locT = state.tile([128, KE, NL], f32r, name=f"xnT{l}",
                                tag="h_locT")
            transpose_to(x_locT, x_loc)
            if l < n_tb - 1:
                xT_full = big.tile([128, KE, T], f32r, name=f"xTf{l}",
                                   tag="hT_full", bufs=2)
                allgather(x_locT, xT_full)

        nc.sync.dma_start(out=out_x.rearrange("b p e -> p b e"), in_=x_loc)


# ---------------- host side ----------------

def _t2(a):
    x = a.shape[0] // 128
    a2 = a.reshape(x, 128, -1).transpose(1, 0, 2)
    return np.ascontiguousarray(a2).astype(np.float32)


def _prep_inputs(input_node, inputad, res, inputtext, linenode, modification,
                 churn, params, n_tb=NTB, n_gg=2):
    f = np.float32
    tok = np.asarray(params["tok_emb"], f)
    tok1 = np.asarray(params["tok_emb1"], f)
    inode = np.asarray(input_node)
    lnode = np.asarray(linenode)
    nodeem = tok[inode[0]]
    x_node = np.concatenate([nodeem, np.asarray(inputtext, f)[0][:, None]], 1)
    lineem = tok1[lnode[0]]
    x_line = np.concatenate(
        [lineem, np.asarray(modification, f)[0][:, None],
         np.asarray(churn, f)[0][:, None]], 1)
    h0 = np.concatenate([x_node, x_line], 0).astype(f)

    mask = np.concatenate([(inode[0] > 0), np.ones(NLLEN, bool)]).astype(f)
    m_tiles = np.ascontiguousarray(mask.reshape(NJB, 128).T).astype(f)
    m32 = np.repeat(m_tiles[:, :, None], 32, axis=2).astype(ml_dtypes.bfloat16)

    adj = np.asarray(inputad, f)

    com = {
        "h0T_full": _t2(np.ascontiguousarray(h0.T)),
        "m_tiles": m_tiles,
        "m32": m32,
        "ident": np.eye(128, dtype=f),
    }
    for g, key in enumerate(["g1", "g2"][:n_gg]):
        gp = params[key]
        W = np.asarray(gp["W"], f)
        a = np.asarray(gp["a"], f)
        gr = gp["gru"]
        Wi = np.asarray(gr["Wi"], f)
        Wh = np.asarray(gr["Wh"], f)
        com[f"g{g}_Wx"] = _t2(np.concatenate([W, W @ a[E:]], 1))
        com[f"g{g}_Wa1"] = _t2(W @ a[:E])
        com[f"g{g}_WiT"] = _t2(np.ascontiguousarray(Wi.T))
        com[f"g{g}_WhT"] = _t2(np.ascontiguousarray(Wh.T))
        com[f"g{g}_bi"] = np.asarray(gr["bi"], f).reshape(1, -1)
        com[f"g{g}_bh"] = np.asarray(gr["bh"], f).reshape(1, -1)
    for l in range(n_tb):
        tb = params["tblocks"][l]
        gv = lambda k: np.asarray(tb[k], f)
        com[f"t{l}_wq"] = _t2(gv("Wq"))
        com[f"t{l}_wk"] = _t2(gv("Wk"))
        com[f"t{l}_wv"] = _t2(gv("Wv"))
        com[f"t{l}_wo"] = _t2(gv("Wo"))
        com[f"t{l}_bq"] = _t2(gv("bq").reshape(E, 1))
        com[f"t{l}_bk"] = _t2(gv("bk").reshape(E, 1))
        com[f"t{l}_bv"] = gv("bv").reshape(1, E)
        com[f"t{l}_bo"] = gv("bo").reshape(1, E)
        com[f"t{l}_w1"] = _t2(gv("W1"))
        com[f"t{l}_b1"] = _t2(gv("b1").reshape(FFD, 1))
        com[f"t{l}_w2"] = _t2(gv("W2")).astype(ml_dtypes.bfloat16)
        com[f"t{l}_b2"] = gv("b2").reshape(1, E)
        for nm, key in [("ln1g", "ln1_g"), ("ln1b", "ln1_b"),
                        ("ln2g", "ln2_g"), ("ln2b", "ln2_b")]:
            com[f"t{l}_{nm}"] = gv(key).reshape(1, E)

    bf_keys = ["h0T_full"]
    for g in range(n_gg):
        bf_keys += [f"g{g}_{s}" for s in
                    ["Wx", "Wa1", "WiT", "WhT", "bi", "bh"]]
    for l in range(n_tb):
        bf_keys += [f"t{l}_{s}" for s in
                    ["wq", "wk", "wv", "wo", "bv", "bo", "w1", "b2"]]
    for k in bf_keys:
        com[k] = com[k].astype(ml_dtypes.bfloat16)

    in_maps = []
    for r in range(R):
        blk = h0[r * NL:(r + 1) * NL]
        d = dict(com)
        d["h0_loc"] = _t2(blk)
        d["h0_locT"] = _t2(np.ascontiguousarray(blk.T)).astype(
            ml_dtypes.bfloat16)
        d["adjT"] = _t2(np.ascontiguousarray(
            adj[r * NL:(r + 1) * NL].T)).astype(ml_dtypes.bfloat16)
        in_maps.append(d)
    return in_maps, h0


_CACHE = {}


def run_device(in_maps, n_tb=NTB, n_gg=2, trace=False):
    key = (n_tb, n_gg)
    if key not in _CACHE:
        _CACHE[key] = build_bass(n_tb, n_gg)
    nc = _CACHE[key]
    return run_bass_kernel_spmd(nc, in_maps, core_ids=list(range(R)),
                                trace=trace)


def kernel(input_node, inputad, res, inputtext, linenode, modification, churn,
           params, _trace=False, _n_tb=NTB, _n_gg=2):
    in_maps, _ = _prep_inputs(input_node, inputad, res, inputtext, linenode,
                              modification, churn, params, _n_tb, _n_gg)
    out = run_device(in_maps, _n_tb, _n_gg, trace=_trace)
    blocks = [np.asarray(out.results[r]["out_x"]).reshape(NL, E)
              for r in range(R)]
    x_full = np.concatenate(blocks, 0)
    kernel._x_full = x_full
    kernel._profile = out

    f = np.float32
    x = x_full[:NLLEN][None]
    wv = np.asarray(params["res2_w"], f)
    bv = np.asarray(params["res2_b"], f)
    logits = (x @ wv + bv).squeeze(-1).astype(f)
    inode = np.asarray(input_node)
    resmask = inode == 2
    logits = np.where(resmask, logits, f(-1e9)).astype(f)
    zmax = logits.max(-1, keepdims=True)
    ez = np.exp(logits - zmax, dtype=f)
    psm = (ez / ez.sum(-1, keepdims=True)).astype(f)
    loss = np.sum(-np.log(np.clip(psm, 1e-10, 1.0)) * np.asarray(res, f),
                  axis=-1).astype(f)
    return loss, psm, x.astype(f)


# revision 13
# speedup vs baseline: 4293.0752x; 4293.0752x over previous
"""Trainium2 Bass kernel for nn_NlEncoder (GNN message passing + transformer).

Sharding: N=2048 rows split across 8 cores (256 rows each). Each layer: every
core computes its row-block of scores/attention against all-gathered key-side
tensors, then local GRU/FFN. AllGather of transposed local blocks between
layers. Embedding gather + tiny output head run on host.

Device layouts per core:
  natural    [128, NB, X]: row n = b*128+p on partitions, features free
  transposed [128, KE, N]: feature e = k*128+p on partitions, rows free
f32 matmuls issue as float32r (full-rate PE) via AP bitcast; the attention
inner loop (scores / exp / att@V) runs in bf16 operands with f32 PSUM accum.
"""

import contextlib
import numpy as np
import ml_dtypes

import concourse.bass as bass
import concourse.bacc as bacc
import concourse.tile as tile
from concourse import mybir
from concourse.bass_utils import run_bass_kernel_spmd

R = 8
T = 2048
NL = T // R          # 256
E = 256
H = 8
D = 32
FFD = 1024
NTB = 5
NJB = T // 128       # 16
KE = E // 128        # 2
NB = NL // 128       # 2
ALPHA = 0.2
NLLEN = 1024
GELU_COMPOSE = False
LN_NEWTON = False

f32 = mybir.dt.float32
f32r = mybir.dt.bfloat16  # matmul operand dtype (PE full rate, FWL)
bf16 = mybir.dt.bfloat16
AF = mybir.ActivationFunctionType
ALU = mybir.AluOpType


def r32(ap):
    assert ap.dtype != f32, f"f32 operand reached matmul: {ap}"
    return ap


def build_bass(n_tb=NTB, n_gg=2):
    # variant flags: GELU_COMPOSE/LN_NEWTON module-level
    nc = bacc.Bacc("TRN2", num_devices=R, name="nlenc")
    with tile.TileContext(nc) as tc:
        _build_body(nc, tc, n_tb, n_gg)
    nc.compile()
    return nc


def _build_body(nc, tc, n_tb, n_gg):
    din = {}

    def inp(name, shape, dtype=f32):
        din[name] = nc.dram_tensor(name, list(shape), dtype,
                                   kind="ExternalInput")
        return din[name]

    def dap(name):
        h = din[name]
        return h[tuple(slice(None) for _ in h.shape)]

    inp("h0T_full", [128, KE, T], f32r)
    inp("h0_loc", [128, NB, E])
    inp("h0_locT", [128, KE, NL], f32r)
    inp("adjT", [128, NJB, NL], bf16)
    inp("m_tiles", [128, NJB])
    inp("m32", [128, NJB, 32], bf16)
    inp("ident", [128, 128])
    for g in range(n_gg):
        inp(f"g{g}_Wx", [128, KE, E + 1], f32r)
        inp(f"g{g}_Wa1", [128, KE, 1], f32r)
        inp(f"g{g}_WiT", [128, KE, 3 * E], f32r)
        inp(f"g{g}_WhT", [128, KE, 3 * E], f32r)
        inp(f"g{g}_bi", [1, 3 * E], f32r)
        inp(f"g{g}_bh", [1, 3 * E], f32r)
    TBW = [("wq", [128, KE, E], f32r), ("wk", [128, KE, E], f32r),
           ("wv", [128, KE, E], f32r), ("wo", [128, KE, E], f32r),
           ("bq", [128, KE, 1], f32), ("bk", [128, KE, 1], f32),
           ("bv", [1, E], f32r), ("bo", [1, E], f32r),
           ("w1", [128, KE, FFD], f32r), ("b1", [128, FFD // 128, 1], f32),
           ("w2", [128, FFD // 128, E], bf16), ("b2", [1, E], f32r),
           ("ln1g", [1, E], f32), ("ln1b", [1, E], f32),
           ("ln2g", [1, E], f32), ("ln2b", [1, E], f32)]
    for l in range(n_tb):
        for nm, shp, dt_ in TBW:
            inp(f"t{l}_{nm}", shp, dt_)

    out_x = nc.dram_tensor("out_x", [NB, 128, E], f32, kind="ExternalOutput")

    ctx = contextlib.ExitStack()
    with ctx:
        sing = ctx.enter_context(tc.tile_pool(name="sing", bufs=1))
        state = ctx.enter_context(tc.tile_pool(name="state", bufs=2))
        big = ctx.enter_context(tc.tile_pool(name="bigsb", bufs=1))
        wpool = ctx.enter_context(tc.tile_pool(name="wpool", bufs=1))
        work = ctx.enter_context(tc.tile_pool(name="work", bufs=1))
        spool = ctx.enter_context(tc.tile_pool(name="spool", bufs=2))
        ps1 = ctx.enter_context(tc.tile_pool(name="ps1", bufs=4, space="PSUM"))
        ps2 = ctx.enter_context(tc.tile_pool(name="ps2", bufs=2, space="PSUM"))
        dram = ctx.enter_context(tc.tile_pool(name="dram", bufs=2,
                                              space="DRAM"))

        def mm(out, lhsT, rhs, **kw):
            nc.tensor.matmul(out, r32(lhsT), r32(rhs), **kw)

        # ---- constants ----
        ident = sing.tile([128, 128], f32, name="identc")
        nc.sync.dma_start(out=ident, in_=dap("ident"))
        ones_row = sing.tile([1, 128], f32r, name="ones_row")
        nc.vector.memset(ones_row, 1.0)
        eps_t = sing.tile([128, 1], f32, name="eps_t")
        nc.vector.memset(eps_t, 1e-5)
        adjT = sing.tile([128, NJB, NL], bf16, name="adjTc")
        nc.sync.dma_start(out=adjT, in_=dap("adjT"))
        m_tiles = sing.tile([128, NJB], f32, name="m_tilesc")
        nc.sync.dma_start(out=m_tiles, in_=dap("m_tiles"))
        m32 = sing.tile([128, NJB, 32], bf16, name="m32c")
        nc.sync.dma_start(out=m32, in_=dap("m32"))

        h_loc = state.tile([128, NB, E], f32, name="h_loc", tag="h_loc")
        h_locT = state.tile([128, KE, NL], f32r, name="h_locT", tag="h_locT")
        hT_full = big.tile([128, KE, T], f32r, name="hT_full", tag="hT_full",
                           bufs=2)
        nc.sync.dma_start(out=h_loc, in_=dap("h0_loc"))
        nc.sync.dma_start(out=h_locT, in_=dap("h0_locT"))
        nc.sync.dma_start(out=hT_full, in_=dap("h0T_full"))

        def cp(i, out, in_):
            if i % 2:
                nc.scalar.copy(out=out, in_=in_)
            else:
                nc.vector.tensor_copy(out=out, in_=in_)

        def transpose_to(dst, src_nat):
            for b in range(NB):
                for k in range(KE):
                    tp = ps1.tile([128, 512], f32, name="tp", tag="ps1")
                    nc.tensor.transpose(
                        tp[:, :128], src_nat[:, b, k * 128:(k + 1) * 128],
                        ident)
                    cp(b * KE + k, dst[:, k, b * 128:(b + 1) * 128],
                       tp[:, :128])

        def allgather(srcT, dstT_full):
            cont = dram.tile([KE, 128, NL], f32r, name="agin", tag="agin")
            gout = dram.tile([R, KE, 128, NL], f32r, name="agout", tag="agout",
                             addr_space="Shared")
            nc.sync.dma_start(out=cont.rearrange("k p n -> p k n"), in_=srcT)
            nc.gpsimd.collective_compute(
                "AllGather", ALU.bypass,
                replica_groups=[list(range(R))],
                ins=[cont.opt()], outs=[gout.opt()],
            )
            for k in range(KE):
                for rh in range(2):
                    nc.sync.dma_start(
                        out=dstT_full[:, k, rh * (T // 2):(rh + 1) * (T // 2)]
                            .rearrange("p (r n) -> p r n", r=R // 2),
                        in_=gout[rh * (R // 2):(rh + 1) * (R // 2), k, :, :]
                            .rearrange("r p n -> p r n"),
                    )

        # ================= GGANN =================
        for g in range(n_gg):
            Wx = wpool.tile([128, KE, E + 1], f32r, name=f"Wx{g}", tag="wsmall",
                            bufs=8)
            Wa1 = wpool.tile([128, KE, 1], f32r, name=f"Wa1{g}", tag="wcol",
                             bufs=8)
            WiT = wpool.tile([128, KE, 3 * E], f32r, name=f"WiT{g}", tag="wbig",
                             bufs=3)
            WhT = wpool.tile([128, KE, 3 * E], f32r, name=f"WhT{g}", tag="wbig",
                             bufs=3)
            bi_r = wpool.tile([1, 3 * E], f32r, name=f"bi{g}", tag="wvec",
                              bufs=4)
            bh_r = wpool.tile([1, 3 * E], f32r, name=f"bh{g}", tag="wvec",
                              bufs=4)
            for nm, tl in [("Wx", Wx), ("Wa1", Wa1), ("WiT", WiT),
                           ("WhT", WhT), ("bi", bi_r), ("bh", bh_r)]:
                nc.sync.dma_start(out=tl, in_=dap(f"g{g}_{nm}"))

            # Wh(+e2) per j-block; Whg bf16 with trailing ones column
            Whg = big.tile([128, NJB, E + 1], bf16, name=f"Whg{g}", tag="bigA")
            e2c = work.tile([128, NJB], f32, name=f"e2c{g}", tag="e2c")
            for jb in range(NJB):
                pw = ps1.tile([128, 512], f32, name="pw", tag="ps1")
                for k in range(KE):
                    mm(pw[:, :E + 1], hT_full[:, k, jb * 128:(jb + 1) * 128],
                       Wx[:, k, :], start=(k == 0), stop=(k == KE - 1))
                cp(jb, Whg[:, jb, :E], pw[:, :E])
                nc.vector.tensor_copy(e2c[:, jb:jb + 1], pw[:, E:E + 1])
                nc.vector.memset(Whg[:, jb, E:E + 1], 1.0)

            pe1 = ps1.tile([128, 512], f32, name="pe1", tag="ps1")
            for k in range(KE):
                mm(pe1[:1, :NL], Wa1[:, k, :], h_locT[:, k, :],
                   start=(k == 0), stop=(k == KE - 1))
            e1row = work.tile([1, NL], f32r, name="e1row", tag="e1row")
            nc.vector.tensor_copy(e1row, pe1[:1, :NL])
            pb = ps1.tile([128, 512], f32, name="pb", tag="ps1")
            mm(pb[:, :NL], ones_row, e1row, start=True, stop=True)
            E1b = work.tile([128, NL], f32, name="E1b", tag="E1b")
            nc.vector.tensor_copy(E1b, pb[:, :NL])

            hp_ps = [ps1.tile([128, 512], f32, name=f"hp{b}", tag="ps1")
                     for b in range(NB)]
            for half in range(2):
                strip = spool.tile([128, NJB // 2, NL], bf16,
                                   name=f"str{g}{half}", tag="ggstrip")
                for j in range(NJB // 2):
                    jb = half * (NJB // 2) + j
                    nc.scalar.activation(
                        out=strip[:, j, :], in_=E1b, func=AF.Prelu,
                        bias=e2c[:, jb:jb + 1], scale=1.0, alpha=ALPHA)
                nc.scalar.activation(out=strip[:, :, :], in_=strip[:, :, :],
                                     func=AF.Exp)
                nc.vector.tensor_mul(
                    strip[:, :, :], strip[:, :, :],
                    adjT[:, half * (NJB // 2):(half + 1) * (NJB // 2), :])
                for j in range(NJB // 2):
                    jb = half * (NJB // 2) + j
                    for b in range(NB):
                        mm(hp_ps[b][:, :E + 1],
                           strip[:, j, b * 128:(b + 1) * 128],
                           Whg[:, jb, :],
                           start=(jb == 0), stop=(jb == NJB - 1),
                           skip_group_check=True)

            h_p = work.tile([128, NB, E], f32, name=f"h_p{g}", tag="h_p")
            for b in range(NB):
                rsum = work.tile([128, 1], f32, name=f"rs{g}{b}", tag="rsum")
                nc.vector.tensor_scalar_add(rsum, hp_ps[b][:, E:E + 1], 1e-30)
                rrec = work.tile([128, 1], f32, name=f"rr{g}{b}", tag="rrec")
                nc.vector.reciprocal(rrec, rsum)
                nc.vector.tensor_scalar_mul(h_p[:, b, :], hp_ps[b][:, :E],
                                            rrec)

            h_pT = work.tile([128, KE, NL], f32r, name=f"h_pT{g}", tag="h_pT")
            transpose_to(h_pT, h_p)
            h_new = state.tile([128, NB, E], f32, name=f"h_new{g}",
                               tag="h_loc")
            for b in range(NB):
                # r/z gates: gi+gh accumulated jointly in one PSUM tile
                prz = ps1.tile([128, 512], f32, name="prz", tag="ps1")
                mm(prz, ones_row, bi_r[:, 0:512], start=True, stop=False,
                   skip_group_check=True)
                mm(prz, ones_row, bh_r[:, 0:512], start=False, stop=False,
                   skip_group_check=True)
                for k in range(KE):
                    mm(prz, h_pT[:, k, b * 128:(b + 1) * 128],
                       WiT[:, k, 0:512], start=False, stop=False,
                       skip_group_check=True)
                    mm(prz, h_locT[:, k, b * 128:(b + 1) * 128],
                       WhT[:, k, 0:512], start=False, stop=(k == KE - 1),
                       skip_group_check=True)
                # n gate: inn and hn separate
                pin = ps1.tile([128, 512], f32, name="pin", tag="ps1")
                phn = ps1.tile([128, 512], f32, name="phn", tag="ps1")
                mm(pin[:, :E], ones_row, bi_r[:, 512:768], start=True,
                   stop=False, skip_group_check=True)
                mm(phn[:, :E], ones_row, bh_r[:, 512:768], start=True,
                   stop=False, skip_group_check=True)
                for k in range(KE):
                    mm(pin[:, :E], h_pT[:, k, b * 128:(b + 1) * 128],
                       WiT[:, k, 512:768], start=False, stop=(k == KE - 1),
                       skip_group_check=True)
                    mm(phn[:, :E], h_locT[:, k, b * 128:(b + 1) * 128],
                       WhT[:, k, 512:768], start=False, stop=(k == KE - 1),
                       skip_group_check=True)
                trz = work.tile([128, 2 * E], f32, name="trz", tag="gr_rz")
                tn = work.tile([128, E], f32, name="tn", tag="gr_n")
                t2 = work.tile([128, E], f32, name="t2", tag="gr_t")
                # sigmoid(x) = 1/(1+exp(-x)) -- keeps ACT in the exp table set
                nc.scalar.activation(out=trz, in_=prz[:, 0:2 * E],
                                     func=AF.Exp, scale=-1.0)
                nc.vector.tensor_scalar_add(trz, trz, 1.0)
                nc.vector.reciprocal(trz, trz)
                tr = trz[:, 0:E]
                tz = trz[:, E:2 * E]
                nc.vector.tensor_mul(tn, tr, phn[:, :E])
                nc.vector.tensor_add(tn, tn, pin[:, :E])
                nc.scalar.activation(out=tn, in_=tn, func=AF.Tanh)
                nc.vector.tensor_sub(t2, h_loc[:, b, :], tn)
                nc.vector.tensor_mul(t2, t2, tz)
                nc.vector.tensor_add(h_new[:, b, :], t2, tn)

            if g == 0:
                ex = work.tile([128, NB, E], f32, name="elu_e", tag="elu_e")
                mk = work.tile([128, NB, E], f32, name="elu_m", tag="elu_m")
                nc.scalar.activation(out=ex[:, :, :], in_=h_new[:, :, :],
                                     func=AF.Exp)
                nc.vector.tensor_scalar(mk[:, :, :], h_new[:, :, :], 0.0,
                                        None, op0=ALU.is_gt)
                nc.vector.tensor_scalar_add(ex[:, :, :], ex[:, :, :], -1.0)
                nc.vector.tensor_sub(h_new[:, :, :], h_new[:, :, :],
                                     ex[:, :, :])
                nc.vector.tensor_mul(h_new[:, :, :], h_new[:, :, :],
                                     mk[:, :, :])
                nc.vector.tensor_add(h_new[:, :, :], h_new[:, :, :],
                                     ex[:, :, :])

            h_loc = h_new
            h_locT = state.tile([128, KE, NL], f32r, name=f"h_nT{g}",
                                tag="h_locT")
            transpose_to(h_locT, h_loc)
            if g < n_gg - 1 or n_tb > 0:
                hT_full = big.tile([128, KE, T], f32r, name=f"hTf{g}",
                                   tag="hT_full", bufs=2)
                allgather(h_locT, hT_full)

        # ================= transformer =================
        x_loc, x_locT, xT_full = h_loc, h_locT, hT_full
        for l in range(n_tb):
            w = {}
            for nm, shp, dt_ in TBW:
                if nm in ("wq", "wk", "wv", "wo"):
                    tg, bf = "wsmall", 8
                elif nm in ("w1", "w2"):
                    tg, bf = "wbig", 3
                elif nm in ("bv", "bo", "b2"):
                    tg, bf = "wvec", 4
                elif nm in ("bq", "bk", "b1"):
                    tg, bf = "wcol", 8
                else:
                    tg, bf = None, None
                if tg is not None:
                    w[nm] = wpool.tile(shp, dt_, name=f"{nm}{l}", tag=tg,
                                       bufs=bf)
                    nc.sync.dma_start(out=w[nm], in_=dap(f"t{l}_{nm}"))
            lnb = {}
            for nm in ["ln1g", "ln1b", "ln2g", "ln2b"]:
                t = wpool.tile([128, E], f32, name=f"{nm}b{l}", tag="lnb",
                               bufs=8)
                src = dap(f"t{l}_{nm}")
                nc.sync.dma_start(
                    out=t,
                    in_=bass.AP(tensor=src.tensor, offset=src.offset,
                                ap=[[0, 128], src.ap[-1]]))
                lnb[nm] = t

            QT = work.tile([128, KE, NL], bf16, name=f"QT{l}", tag="QT")
            for t in range(KE):
                pq = ps1.tile([128, 512], f32, name="pq", tag="ps1")
                for k in range(KE):
                    mm(pq[:, :NL], w["wq"][:, k, t * 128:(t + 1) * 128],
                       x_locT[:, k, :], start=(k == 0), stop=(k == KE - 1))
                nc.vector.tensor_scalar(
                    QT[:, t, :], pq[:, :NL], w["bq"][:, t, :],
                    float(1.0 / np.sqrt(D)), op0=ALU.add, op1=ALU.mult)

            KT = big.tile([128, KE, T], bf16, name=f"KT{l}", tag="bigA")
            for t in range(KE):
                for jc in range(T // 512):
                    pk = ps1.tile([128, 512], f32, name="pk", tag="ps1")
                    for k in range(KE):
                        mm(pk, w["wk"][:, k, t * 128:(t + 1) * 128],
                           xT_full[:, k, jc * 512:(jc + 1) * 512],
                           start=(k == 0), stop=(k == KE - 1))
                    nc.vector.tensor_scalar_add(
                        KT[:, t, jc * 512:(jc + 1) * 512], pk,
                        w["bk"][:, t, :])

            Vm = big.tile([128, NJB, E], bf16, name=f"Vm{l}", tag="bigB")
            for jb in range(NJB):
                pv = ps1.tile([128, 512], f32, name="pv", tag="ps1")
                mm(pv[:, :E], ones_row, w["bv"], start=True, stop=False,
                   skip_group_check=True)
                for k in range(KE):
                    mm(pv[:, :E], xT_full[:, k, jb * 128:(jb + 1) * 128],
                       w["wv"][:, k, :], start=False, stop=(k == KE - 1),
                       skip_group_check=True)
                nc.vector.tensor_scalar_mul(Vm[:, jb, :], pv[:, :E],
                                            m_tiles[:, jb:jb + 1])

            OTn = work.tile([128, KE, NL], f32r, name=f"OTn{l}", tag="OTn")
            for q in range(2):
                strips = [spool.tile([128, NJB, NL], bf16,
                                     name=f"u{l}{q}{h4}", tag="ustrip",
                                     bufs=4)
                          for h4 in range(4)]
                for jq in range(4):
                    for hp in range(2):
                        scs = [ps2.tile([128, 1024], f32, name="sc",
                                        tag="ps2") for _ in range(2)]
                        for j in range(4):
                            jb = jq * 4 + j
                            for i in range(2):
                                h4 = hp * 2 + i
                                mm(scs[i][:, j * 256:(j + 1) * 256],
                                   KT[h4 * 32:(h4 + 1) * 32, q,
                                      jb * 128:(jb + 1) * 128],
                                   QT[h4 * 32:(h4 + 1) * 32, q, :],
                                   start=True, stop=True,
                                   tile_position=(h4 * 32, 0),
                                   skip_group_check=True)
                        for i in range(2):
                            h4 = hp * 2 + i
                            nc.scalar.activation(
                                out=strips[h4][:, jq * 4:(jq + 1) * 4, :],
                                in_=scs[i], func=AF.Exp)
                OT_ps = ps1.tile([128, 512], f32, name="otps", tag="ps1")
                RS_ps = ps1.tile([128, 512], f32, name="rsps", tag="ps1")
                for jb in range(NJB):
                    for h4 in range(4):
                        h = q * 4 + h4
                        mm(OT_ps[h4 * 32:(h4 + 1) * 32, :NL],
                           Vm[:, jb, h * 32:(h + 1) * 32],
                           strips[h4][:, jb, :],
                           start=(jb == 0 and h4 == 0), stop=(jb == NJB - 1),
                           tile_position=(0, h4 * 32), skip_group_check=True)
                        mm(RS_ps[h4 * 32:(h4 + 1) * 32, :NL],
                           m32[:, jb, :], strips[h4][:, jb, :],
                           start=(jb == 0 and h4 == 0), stop=(jb == NJB - 1),
                           tile_position=(0, h4 * 32), skip_group_check=True)
                rrec = work.tile([128, NL], f32, name="rrec", tag="rrec")
                nc.vector.reciprocal(rrec, RS_ps[:, :NL])
                nc.vector.tensor_mul(OTn[:, q, :], OT_ps[:, :NL], rrec)

            x2 = work.tile([128, NB, E], f32, name=f"x2{l}", tag="x2")
            for b in range(NB):
                po = ps1.tile([128, 512], f32, name="po", tag="ps1")
                mm(po[:, :E], ones_row, w["bo"], start=True, stop=False,
                   skip_group_check=True)
                for q in range(KE):
                    mm(po[:, :E], OTn[:, q, b * 128:(b + 1) * 128],
                       w["wo"][:, q, :], start=False, stop=(q == KE - 1),
                       skip_group_check=True)
                nc.vector.tensor_add(x2[:, b, :], po[:, :E], x_loc[:, b, :])

            def layernorm(dst, src, gname, bname):
                mvb = work.tile([128, NB, 2], f32, name="mvb", tag="ln_mv")
                for b in range(NB):
                    st = work.tile([128, 6], f32, name="st", tag="ln_st")
                    nc.vector.bn_stats(out=st, in_=src[:, b, :])
                    nc.vector.bn_aggr(out=mvb[:, b, :], in_=st)
                # rstd = rsqrt(var+eps) on DVE only (bf16 bit-trick + Newton)
                ve = work.tile([128, NB], f32, name="ve", tag="ln_ve")
                y = work.tile([128, NB], f32, name="lny", tag="ln_y")
                if not LN_NEWTON:
                    sdq = work.tile([128, NB], f32, name="sdq", tag="ln_sd")
                    nc.vector.tensor_scalar_add(ve, mvb[:, :, 1], 1e-5)
                    nc.scalar.activation(out=sdq, in_=ve, func=AF.Sqrt)
                    nc.vector.reciprocal(y, sdq)
                else:
                    nc.vector.tensor_scalar_add(ve, mvb[:, :, 1], 1e-5)
                if LN_NEWTON:
                    vb = work.tile([128, NB], bf16, name="vb", tag="ln_vb")
                    nc.vector.tensor_copy(vb, ve)
                    sh = work.tile([128, NB], mybir.dt.int16, name="sh",
                                   tag="ln_sh")
                    nc.vector.tensor_scalar(sh, vb.bitcast(mybir.dt.int16), 1,
                                            None,
                                            op0=ALU.logical_shift_right)
                    nc.vector.tensor_scalar(sh, sh, -1, 24375, op0=ALU.mult,
                                            op1=ALU.add)
                    tq = work.tile([128, NB], f32, name="lnt", tag="ln_t")
                    nc.vector.tensor_copy(y, sh.bitcast(bf16))
                    for _ in range(3):
                        nc.vector.tensor_mul(tq, y, y)
                        nc.vector.tensor_mul(tq, tq, ve)
                        nc.vector.tensor_scalar(tq, tq, -0.5, 1.5,
                                                op0=ALU.mult, op1=ALU.add)
                        nc.vector.tensor_mul(y, y, tq)
                for b in range(NB):
                    nc.vector.tensor_scalar(dst[:, b, :], src[:, b, :],
                                            mvb[:, b, 0:1], y[:, b:b + 1],
                                            op0=ALU.subtract, op1=ALU.mult)
                    nc.vector.tensor_mul(dst[:, b, :], dst[:, b, :],
                                         lnb[gname])
                    nc.vector.tensor_add(dst[:, b, :], dst[:, b, :],
                                         lnb[bname])

            x_ln = work.tile([128, NB, E], f32, name=f"xln{l}", tag="x_ln")
            layernorm(x_ln, x2, "ln1g", "ln1b")

            x_lnT = work.tile([128, KE, NL], f32r, name=f"xlnT{l}",
                              tag="x_lnT")
            transpose_to(x_lnT, x_ln)
            f1 = big.tile([128, FFD // 128, NL], bf16, name=f"f1{l}",
                          tag="bigB")
            C1 = 0.7978845608028654
            CA = C1 * 0.044715
            for mb in range(FFD // 128):
                pf = ps1.tile([128, 512], f32, name="pf", tag="ps1")
                for k in range(KE):
                    mm(pf[:, :NL], w["w1"][:, k, mb * 128:(mb + 1) * 128],
                       x_lnT[:, k, :], start=(k == 0), stop=(k == KE - 1))
                if not GELU_COMPOSE:
                    nc.scalar.activation(out=f1[:, mb, :], in_=pf[:, :NL],
                                         func=AF.Gelu_apprx_tanh,
                                         bias=w["b1"][:, mb, :], scale=1.0)
                else:
                    # gelu_tanh(h) = 0.5 h (1 + tanh(C1 h + CA h^3))
                    hb = work.tile([128, NL], f32, name="hb", tag="ff_h", bufs=3)
                    ug = work.tile([128, NL], f32, name="ug", tag="ff_u", bufs=3)
                    nc.vector.tensor_scalar_add(hb, pf[:, :NL],
                                                w["b1"][:, mb, :])
                    nc.vector.tensor_mul(ug, hb, hb)
                    nc.vector.tensor_scalar(ug, ug, CA, C1, op0=ALU.mult,
                                            op1=ALU.add)
                    nc.vector.tensor_mul(ug, ug, hb)
                    nc.scalar.activation(out=ug, in_=ug, func=AF.Tanh)
                    nc.vector.tensor_scalar(ug, ug, 1.0, 0.5, op0=ALU.add,
                                            op1=ALU.mult)
                    nc.vector.tensor_mul(f1[:, mb, :], ug, hb)
            x3 = work.tile([128, NB, E], f32, name=f"x3{l}", tag="x3")
            for b in range(NB):
                pf2 = ps1.tile([128, 512], f32, name="pf2", tag="ps1")
                mm(pf2[:, :E], ones_row, w["b2"], start=True, stop=False,
                   skip_group_check=True)
                for km in range(FFD // 128):
                    mm(pf2[:, :E], f1[:, km, b * 128:(b + 1) * 128],
                       w["w2"][:, km, :], start=False,
                       stop=(km == FFD // 128 - 1), skip_group_check=True)
                nc.vector.tensor_add(x3[:, b, :], pf2[:, :E], x_ln[:, b, :])

            x_new = state.tile([128, NB, E], f32, name=f"xn{l}", tag="h_loc")
            layernorm(x_new, x3, "ln2g", "ln2b")

            x_loc = x_new
            x_locT = state.tile([128, KE, NL], f32r, name=f"xnT{l}",
                                tag="h_locT")
            transpose_to(x_locT, x_loc)
            if l < n_tb - 1:
                xT_full = big.tile([128, KE, T], f32r, name=f"xTf{l}",
                                   tag="hT_full", bufs=2)
                allgather(x_locT, xT_full)

        nc.sync.dma_start(out=out_x.rearrange("b p e -> p b e"), in_=x_loc)


# ---------------- host side ----------------

def _t2(a):
    x = a.shape[0] // 128
    a2 = a.reshape(x, 128, -1).transpose(1, 0, 2)
    return np.ascontiguousarray(a2).astype(np.float32)


def _prep_inputs(input_node, inputad, res, inputtext, linenode, modification,
                 churn, params, n_tb=NTB, n_gg=2):
    f = np.float32
    tok = np.asarray(params["tok_emb"], f)
    tok1 = np.asarray(params["tok_emb1"], f)
    inode = np.asarray(input_node)
    lnode = np.asarray(linenode)
    nodeem = tok[inode[0]]
    x_node = np.concatenate([nodeem, np.asarray(inputtext, f)[0][:, None]], 1)
    lineem = tok1[lnode[0]]
    x_line = np.concatenate(
        [lineem, np.asarray(modification, f)[0][:, None],
         np.asarray(churn, f)[0][:, None]], 1)
    h0 = np.concatenate([x_node, x_line], 0).astype(f)

    mask = np.concatenate([(inode[0] > 0), np.ones(NLLEN, bool)]).astype(f)
    m_tiles = np.ascontiguousarray(mask.reshape(NJB, 128).T).astype(f)
    m32 = np.repeat(m_tiles[:, :, None], 32, axis=2).astype(ml_dtypes.bfloat16)

    adj = np.asarray(inputad, f)

    com = {
        "h0T_full": _t2(np.ascontiguousarray(h0.T)),
        "m_tiles": m_tiles,
        "m32": m32,
        "ident": np.eye(128, dtype=f),
    }
    for g, key in enumerate(["g1", "g2"][:n_gg]):
        gp = params[key]
        W = np.asarray(gp["W"], f)
        a = np.asarray(gp["a"], f)
        gr = gp["gru"]
        Wi = np.asarray(gr["Wi"], f)
        Wh = np.asarray(gr["Wh"], f)
        com[f"g{g}_Wx"] = _t2(np.concatenate([W, W @ a[E:]], 1))
        com[f"g{g}_Wa1"] = _t2(W @ a[:E])
        com[f"g{g}_WiT"] = _t2(np.ascontiguousarray(Wi.T))
        com[f"g{g}_WhT"] = _t2(np.ascontiguousarray(Wh.T))
        com[f"g{g}_bi"] = np.asarray(gr["bi"], f).reshape(1, -1)
        com[f"g{g}_bh"] = np.asarray(gr["bh"], f).reshape(1, -1)
    for l in range(n_tb):
        tb = params["tblocks"][l]
        gv = lambda k: np.asarray(tb[k], f)
        com[f"t{l}_wq"] = _t2(gv("Wq"))
        com[f"t{l}_wk"] = _t2(gv("Wk"))
        com[f"t{l}_wv"] = _t2(gv("Wv"))
        com[f"t{l}_wo"] = _t2(gv("Wo"))
        com[f"t{l}_bq"] = _t2(gv("bq").reshape(E, 1))
        com[f"t{l}_bk"] = _t2(gv("bk").reshape(E, 1))
        com[f"t{l}_bv"] = gv("bv").reshape(1, E)
        com[f"t{l}_bo"] = gv("bo").reshape(1, E)
        com[f"t{l}_w1"] = _t2(gv("W1"))
        com[f"t{l}_b1"] = _t2(gv("b1").reshape(FFD, 1))
        com[f"t{l}_w2"] = _t2(gv("W2")).astype(ml_dtypes.bfloat16)
        com[f"t{l}_b2"] = gv("b2").reshape(1, E)
        for nm, key in [("ln1g", "ln1_g"), ("ln1b", "ln1_b"),
                        ("ln2g", "ln2_g"), ("ln2b", "ln2_b")]:
            com[f"t{l}_{nm}"] = gv(key).reshape(1, E)

    bf_keys = ["h0T_full"]
    for g in range(n_gg):
        bf_keys += [f"g{g}_{s}" for s in
                    ["Wx", "Wa1", "WiT", "WhT", "bi", "bh"]]
    for l in range(n_tb):
        bf_keys += [f"t{l}_{s}" for s in
                    ["wq", "wk", "wv", "wo", "bv", "bo", "w1", "b2"]]
    for k in bf_keys:
        com[k] = com[k].astype(ml_dtypes.bfloat16)

    in_maps = []
    for r in range(R):
        blk = h0[r * NL:(r + 1) * NL]
        d = dict(com)
        d["h0_loc"] = _t2(blk)
        d["h0_locT"] = _t2(np.ascontiguousarray(blk.T)).astype(
            ml_dtypes.bfloat16)
        d["adjT"] = _t2(np.ascontiguousarray(
            adj[r * NL:(r + 1) * NL].T)).astype(ml_dtypes.bfloat16)
        in_maps.append(d)
    return in_maps, h0


_CACHE = {}


def run_device(in_maps, n_tb=NTB, n_gg=2, trace=False):
    key = (n_tb, n_gg)
    if key not in _CACHE:
        _CACHE[key] = build_bass(n_tb, n_gg)
    nc = _CACHE[key]
    return run_bass_kernel_spmd(nc, in_maps, core_ids=list(range(R)),
                                trace=trace)


def kernel(input_node, inputad, res, inputtext, linenode, modification, churn,
           params, _trace=False, _n_tb=NTB, _n_gg=2):
    in_maps, _ = _prep_inputs(input_node, inputad, res, inputtext, linenode,
                              modification, churn, params, _n_tb, _n_gg)
    out = run_device(in_maps, _n_tb, _n_gg, trace=_trace)
    blocks = [np.asarray(out.results[r]["out_x"]).reshape(NL, E)
              for r in range(R)]
    x_full = np.concatenate(blocks, 0)
    kernel._x_full = x_full
    kernel._profile = out

    f = np.float32
    x = x_full[:NLLEN][None]
    wv = np.asarray(params["res2_w"], f)
    bv = np.asarray(params["res2_b"], f)
    logits = (x @ wv + bv).squeeze(-1).astype(f)
    inode = np.asarray(input_node)
    resmask = inode == 2
    logits = np.where(resmask, logits, f(-1e9)).astype(f)
    zmax = logits.max(-1, keepdims=True)
    ez = np.exp(logits - zmax, dtype=f)
    psm = (ez / ez.sum(-1, keepdims=True)).astype(f)
    loss = np.sum(-np.log(np.clip(psm, 1e-10, 1.0)) * np.asarray(res, f),
                  axis=-1).astype(f)
    return loss, psm, x.astype(f)


# revision 14
# speedup vs baseline: 4302.5927x; 1.0022x over previous
"""Trainium2 Bass kernel for nn_NlEncoder (GNN message passing + transformer).

Sharding: N=2048 rows split across 8 cores (256 rows each). Each layer: every
core computes its row-block of scores/attention against all-gathered key-side
tensors, then local GRU/FFN. AllGather of transposed local blocks between
layers. Embedding gather + tiny output head run on host.

Device layouts per core:
  natural    [128, NB, X]: row n = b*128+p on partitions, features free
  transposed [128, KE, N]: feature e = k*128+p on partitions, rows free
f32 matmuls issue as float32r (full-rate PE) via AP bitcast; the attention
inner loop (scores / exp / att@V) runs in bf16 operands with f32 PSUM accum.
"""

import contextlib
import numpy as np
import ml_dtypes

import concourse.bass as bass
import concourse.bacc as bacc
import concourse.tile as tile
from concourse import mybir
from concourse.bass_utils import run_bass_kernel_spmd

R = 8
T = 2048
NL = T // R          # 256
E = 256
H = 8
D = 32
FFD = 1024
NTB = 5
NJB = T // 128       # 16
KE = E // 128        # 2
NB = NL // 128       # 2
ALPHA = 0.2
NLLEN = 1024
GELU_COMPOSE = False
LN_NEWTON = False

f32 = mybir.dt.float32
f32r = mybir.dt.bfloat16  # matmul operand dtype (PE full rate, FWL)
bf16 = mybir.dt.bfloat16
AF = mybir.ActivationFunctionType
ALU = mybir.AluOpType


def r32(ap):
    assert ap.dtype != f32, f"f32 operand reached matmul: {ap}"
    return ap


def build_bass(n_tb=NTB, n_gg=2):
    # variant flags: GELU_COMPOSE/LN_NEWTON module-level
    nc = bacc.Bacc("TRN2", num_devices=R, name="nlenc")
    with tile.TileContext(nc) as tc:
        _build_body(nc, tc, n_tb, n_gg)
    nc.compile()
    return nc


def _build_body(nc, tc, n_tb, n_gg):
    din = {}

    def inp(name, shape, dtype=f32):
        din[name] = nc.dram_tensor(name, list(shape), dtype,
                                   kind="ExternalInput")
        return din[name]

    def dap(name):
        h = din[name]
        return h[tuple(slice(None) for _ in h.shape)]

    inp("h0T_full", [128, KE, T], f32r)
    inp("h0_loc", [128, NB, E])
    inp("h0_locT", [128, KE, NL], f32r)
    inp("adjT", [128, NJB, NL], bf16)
    inp("m_tiles", [128, NJB])
    inp("m32", [128, NJB, 32], bf16)
    inp("ident", [128, 128])
    for g in range(n_gg):
        inp(f"g{g}_Wx", [128, KE, E + 1], f32r)
        inp(f"g{g}_Wa1", [128, KE, 1], f32r)
        inp(f"g{g}_WiT", [128, KE, 3 * E], f32r)
        inp(f"g{g}_WhT", [128, KE, 3 * E], f32r)
        inp(f"g{g}_bi", [1, 3 * E], f32r)
        inp(f"g{g}_bh", [1, 3 * E], f32r)
    TBW = [("wq", [128, KE, E], f32r), ("wk", [128, KE, E], f32r),
           ("wv", [128, KE, E], f32r), ("wo", [128, KE, E], f32r),
           ("bq", [128, KE, 1], f32), ("bk", [128, KE, 1], f32),
           ("bv", [1, E], f32r), ("bo", [1, E], f32r),
           ("w1", [128, KE, FFD], f32r), ("b1", [128, FFD // 128, 1], f32),
           ("w2", [128, FFD // 128, E], bf16), ("b2", [1, E], f32r),
           ("ln1g", [1, E], f32), ("ln1b", [1, E], f32),
           ("ln2g", [1, E], f32), ("ln2b", [1, E], f32)]
    for l in range(n_tb):
        for nm, shp, dt_ in TBW:
            inp(f"t{l}_{nm}", shp, dt_)

    out_x = nc.dram_tensor("out_x", [NB, 128, E], f32, kind="ExternalOutput")

    ctx = contextlib.ExitStack()
    with ctx:
        sing = ctx.enter_context(tc.tile_pool(name="sing", bufs=1))
        state = ctx.enter_context(tc.tile_pool(name="state", bufs=2))
        big = ctx.enter_context(tc.tile_pool(name="bigsb", bufs=1))
        wpool = ctx.enter_context(tc.tile_pool(name="wpool", bufs=1))
        work = ctx.enter_context(tc.tile_pool(name="work", bufs=1))
        spool = ctx.enter_context(tc.tile_pool(name="spool", bufs=2))
        ps1 = ctx.enter_context(tc.tile_pool(name="ps1", bufs=4, space="PSUM"))
        ps2 = ctx.enter_context(tc.tile_pool(name="ps2", bufs=2, space="PSUM"))
        dram = ctx.enter_context(tc.tile_pool(name="dram", bufs=2,
                                              space="DRAM"))

        def mm(out, lhsT, rhs, **kw):
            nc.tensor.matmul(out, r32(lhsT), r32(rhs), **kw)

        # ---- constants ----
        ident = sing.tile([128, 128], f32, name="identc")
        nc.sync.dma_start(out=ident, in_=dap("ident"))
        ones_row = sing.tile([1, 128], f32r, name="ones_row")
        nc.vector.memset(ones_row, 1.0)
        eps_t = sing.tile([128, 1], f32, name="eps_t")
        nc.vector.memset(eps_t, 1e-5)
        adjT = sing.tile([128, NJB, NL], bf16, name="adjTc")
        nc.sync.dma_start(out=adjT, in_=dap("adjT"))
        m_tiles = sing.tile([128, NJB], f32, name="m_tilesc")
        nc.sync.dma_start(out=m_tiles, in_=dap("m_tiles"))
        m32 = sing.tile([128, NJB, 32], bf16, name="m32c")
        nc.sync.dma_start(out=m32, in_=dap("m32"))

        h_loc = state.tile([128, NB, E], f32, name="h_loc", tag="h_loc")
        h_locT = state.tile([128, KE, NL], f32r, name="h_locT", tag="h_locT")
        hT_full = big.tile([128, KE, T], f32r, name="hT_full", tag="hT_full",
                           bufs=2)
        nc.sync.dma_start(out=h_loc, in_=dap("h0_loc"))
        nc.sync.dma_start(out=h_locT, in_=dap("h0_locT"))
        nc.sync.dma_start(out=hT_full, in_=dap("h0T_full"))

        def cp(i, out, in_):
            if i % 2:
                nc.scalar.copy(out=out, in_=in_)
            else:
                nc.vector.tensor_copy(out=out, in_=in_)

        def transpose_to(dst, src_nat):
            for b in range(NB):
                for k in range(KE):
                    tp = ps1.tile([128, 512], f32, name="tp", tag="ps1")
                    nc.tensor.transpose(
                        tp[:, :128], src_nat[:, b, k * 128:(k + 1) * 128],
                        ident)
                    cp(b * KE + k, dst[:, k, b * 128:(b + 1) * 128],
                       tp[:, :128])

        def allgather(srcT, dstT_full):
            cont = dram.tile([KE, 128, NL], f32r, name="agin", tag="agin")
            gout = dram.tile([R, KE, 128, NL], f32r, name="agout", tag="agout",
                             addr_space="Shared")
            nc.sync.dma_start(out=cont.rearrange("k p n -> p k n"), in_=srcT)
            nc.gpsimd.collective_compute(
                "AllGather", ALU.bypass,
                replica_groups=[list(range(R))],
                ins=[cont.opt()], outs=[gout.opt()],
            )
            for k in range(KE):
                for rh in range(4):
                    nc.sync.dma_start(
                        out=dstT_full[:, k, rh * (T // 4):(rh + 1) * (T // 4)]
                            .rearrange("p (r n) -> p r n", r=R // 4),
                        in_=gout[rh * (R // 4):(rh + 1) * (R // 4), k, :, :]
                            .rearrange("r p n -> p r n"),
                    )

        # ================= GGANN =================
        for g in range(n_gg):
            Wx = wpool.tile([128, KE, E + 1], f32r, name=f"Wx{g}", tag="wsmall",
                            bufs=8)
            Wa1 = wpool.tile([128, KE, 1], f32r, name=f"Wa1{g}", tag="wcol",
                             bufs=8)
            WiT = wpool.tile([128, KE, 3 * E], f32r, name=f"WiT{g}", tag="wbig",
                             bufs=3)
            WhT = wpool.tile([128, KE, 3 * E], f32r, name=f"WhT{g}", tag="wbig",
                             bufs=3)
            bi_r = wpool.tile([1, 3 * E], f32r, name=f"bi{g}", tag="wvec",
                              bufs=4)
            bh_r = wpool.tile([1, 3 * E], f32r, name=f"bh{g}", tag="wvec",
                              bufs=4)
            for nm, tl in [("Wx", Wx), ("Wa1", Wa1), ("WiT", WiT),
                           ("WhT", WhT), ("bi", bi_r), ("bh", bh_r)]:
                nc.sync.dma_start(out=tl, in_=dap(f"g{g}_{nm}"))

            # Wh(+e2) per j-block; Whg bf16 with trailing ones column
            Whg = big.tile([128, NJB, E + 1], bf16, name=f"Whg{g}", tag="bigA")
            e2c = work.tile([128, NJB], f32, name=f"e2c{g}", tag="e2c")
            for jb in range(NJB):
                pw = ps1.tile([128, 512], f32, name="pw", tag="ps1")
                for k in range(KE):
                    mm(pw[:, :E + 1], hT_full[:, k, jb * 128:(jb + 1) * 128],
                       Wx[:, k, :], start=(k == 0), stop=(k == KE - 1))
                cp(jb, Whg[:, jb, :E], pw[:, :E])
                nc.vector.tensor_copy(e2c[:, jb:jb + 1], pw[:, E:E + 1])
                nc.vector.memset(Whg[:, jb, E:E + 1], 1.0)

            pe1 = ps1.tile([128, 512], f32, name="pe1", tag="ps1")
            for k in range(KE):
                mm(pe1[:1, :NL], Wa1[:, k, :], h_locT[:, k, :],
                   start=(k == 0), stop=(k == KE - 1))
            e1row = work.tile([1, NL], f32r, name="e1row", tag="e1row")
            nc.vector.tensor_copy(e1row, pe1[:1, :NL])
            pb = ps1.tile([128, 512], f32, name="pb", tag="ps1")
            mm(pb[:, :NL], ones_row, e1row, start=True, stop=True)
            E1b = work.tile([128, NL], f32, name="E1b", tag="E1b")
            nc.vector.tensor_copy(E1b, pb[:, :NL])

            hp_ps = [ps1.tile([128, 512], f32, name=f"hp{b}", tag="ps1")
                     for b in range(NB)]
            for half in range(2):
                strip = spool.tile([128, NJB // 2, NL], bf16,
                                   name=f"str{g}{half}", tag="ggstrip")
                for j in range(NJB // 2):
                    jb = half * (NJB // 2) + j
                    nc.scalar.activation(
                        out=strip[:, j, :], in_=E1b, func=AF.Prelu,
                        bias=e2c[:, jb:jb + 1], scale=1.0, alpha=ALPHA)
                nc.scalar.activation(out=strip[:, :, :], in_=strip[:, :, :],
                                     func=AF.Exp)
                nc.vector.tensor_mul(
                    strip[:, :, :], strip[:, :, :],
                    adjT[:, half * (NJB // 2):(half + 1) * (NJB // 2), :])
                for j in range(NJB // 2):
                    jb = half * (NJB // 2) + j
                    for b in range(NB):
                        mm(hp_ps[b][:, :E + 1],
                           strip[:, j, b * 128:(b + 1) * 128],
                           Whg[:, jb, :],
                           start=(jb == 0), stop=(jb == NJB - 1),
                           skip_group_check=True)

            h_p = work.tile([128, NB, E], f32, name=f"h_p{g}", tag="h_p")
            for b in range(NB):
                rsum = work.tile([128, 1], f32, name=f"rs{g}{b}", tag="rsum")
                nc.vector.tensor_scalar_add(rsum, hp_ps[b][:, E:E + 1], 1e-30)
                rrec = work.tile([128, 1], f32, name=f"rr{g}{b}", tag="rrec")
                nc.vector.reciprocal(rrec, rsum)
                nc.vector.tensor_scalar_mul(h_p[:, b, :], hp_ps[b][:, :E],
                                            rrec)

            h_pT = work.tile([128, KE, NL], f32r, name=f"h_pT{g}", tag="h_pT")
            transpose_to(h_pT, h_p)
            h_new = state.tile([128, NB, E], f32, name=f"h_new{g}",
                               tag="h_loc")
            for b in range(NB):
                # r/z gates: gi+gh accumulated jointly in one PSUM tile
                prz = ps1.tile([128, 512], f32, name="prz", tag="ps1")
                mm(prz, ones_row, bi_r[:, 0:512], start=True, stop=False,
                   skip_group_check=True)
                mm(prz, ones_row, bh_r[:, 0:512], start=False, stop=False,
                   skip_group_check=True)
                for k in range(KE):
                    mm(prz, h_pT[:, k, b * 128:(b + 1) * 128],
                       WiT[:, k, 0:512], start=False, stop=False,
                       skip_group_check=True)
                    mm(prz, h_locT[:, k, b * 128:(b + 1) * 128],
                       WhT[:, k, 0:512], start=False, stop=(k == KE - 1),
                       skip_group_check=True)
                # n gate: inn and hn separate
                pin = ps1.tile([128, 512], f32, name="pin", tag="ps1")
                phn = ps1.tile([128, 512], f32, name="phn", tag="ps1")
                mm(pin[:, :E], ones_row, bi_r[:, 512:768], start=True,
                   stop=False, skip_group_check=True)
                mm(phn[:, :E], ones_row, bh_r[:, 512:768], start=True,
                   stop=False, skip_group_check=True)
                for k in range(KE):
                    mm(pin[:, :E], h_pT[:, k, b * 128:(b + 1) * 128],
                       WiT[:, k, 512:768], start=False, stop=(k == KE - 1),
                       skip_group_check=True)
                    mm(phn[:, :E], h_locT[:, k, b * 128:(b + 1) * 128],
                       WhT[:, k, 512:768], start=False, stop=(k == KE - 1),
                       skip_group_check=True)
                trz = work.tile([128, 2 * E], f32, name="trz", tag="gr_rz")
                tn = work.tile([128, E], f32, name="tn", tag="gr_n")
                t2 = work.tile([128, E], f32, name="t2", tag="gr_t")
                # sigmoid(x) = 1/(1+exp(-x)) -- keeps ACT in the exp table set
                nc.scalar.activation(out=trz, in_=prz[:, 0:2 * E],
                                     func=AF.Exp, scale=-1.0)
                nc.vector.tensor_scalar_add(trz, trz, 1.0)
                nc.vector.reciprocal(trz, trz)
                tr = trz[:, 0:E]
                tz = trz[:, E:2 * E]
                nc.vector.tensor_mul(tn, tr, phn[:, :E])
                nc.vector.tensor_add(tn, tn, pin[:, :E])
                nc.scalar.activation(out=tn, in_=tn, func=AF.Tanh)
                nc.vector.tensor_sub(t2, h_loc[:, b, :], tn)
                nc.vector.tensor_mul(t2, t2, tz)
                nc.vector.tensor_add(h_new[:, b, :], t2, tn)

            if g == 0:
                ex = work.tile([128, NB, E], f32, name="elu_e", tag="elu_e")
                mk = work.tile([128, NB, E], f32, name="elu_m", tag="elu_m")
                nc.scalar.activation(out=ex[:, :, :], in_=h_new[:, :, :],
                                     func=AF.Exp)
                nc.vector.tensor_scalar(mk[:, :, :], h_new[:, :, :], 0.0,
                                        None, op0=ALU.is_gt)
                nc.vector.tensor_scalar_add(ex[:, :, :], ex[:, :, :], -1.0)
                nc.vector.tensor_sub(h_new[:, :, :], h_new[:, :, :],
                                     ex[:, :, :])
                nc.vector.tensor_mul(h_new[:, :, :], h_new[:, :, :],
                                     mk[:, :, :])
                nc.vector.tensor_add(h_new[:, :, :], h_new[:, :, :],
                                     ex[:, :, :])

            h_loc = h_new
            h_locT = state.tile([128, KE, NL], f32r, name=f"h_nT{g}",
                                tag="h_locT")
            transpose_to(h_locT, h_loc)
            if g < n_gg - 1 or n_tb > 0:
                hT_full = big.tile([128, KE, T], f32r, name=f"hTf{g}",
                                   tag="hT_full", bufs=2)
                allgather(h_locT, hT_full)

        # ================= transformer =================
        x_loc, x_locT, xT_full = h_loc, h_locT, hT_full
        for l in range(n_tb):
            w = {}
            for nm, shp, dt_ in TBW:
                if nm in ("wq", "wk", "wv", "wo"):
                    tg, bf = "wsmall", 8
                elif nm in ("w1", "w2"):
                    tg, bf = "wbig", 3
                elif nm in ("bv", "bo", "b2"):
                    tg, bf = "wvec", 4
                elif nm in ("bq", "bk", "b1"):
                    tg, bf = "wcol", 8
                else:
                    tg, bf = None, None
                if tg is not None:
                    w[nm] = wpool.tile(shp, dt_, name=f"{nm}{l}", tag=tg,
                                       bufs=bf)
                    nc.sync.dma_start(out=w[nm], in_=dap(f"t{l}_{nm}"))
            lnb = {}
            for nm in ["ln1g", "ln1b", "ln2g", "ln2b"]:
                t = wpool.tile([128, E], f32, name=f"{nm}b{l}", tag="lnb",
                               bufs=8)
                src = dap(f"t{l}_{nm}")
                nc.sync.dma_start(
                    out=t,
                    in_=bass.AP(tensor=src.tensor, offset=src.offset,
                                ap=[[0, 128], src.ap[-1]]))
                lnb[nm] = t

            QT = work.tile([128, KE, NL], bf16, name=f"QT{l}", tag="QT")
            for t in range(KE):
                pq = ps1.tile([128, 512], f32, name="pq", tag="ps1")
                for k in range(KE):
                    mm(pq[:, :NL], w["wq"][:, k, t * 128:(t + 1) * 128],
                       x_locT[:, k, :], start=(k == 0), stop=(k == KE - 1))
                nc.vector.tensor_scalar(
                    QT[:, t, :], pq[:, :NL], w["bq"][:, t, :],
                    float(1.0 / np.sqrt(D)), op0=ALU.add, op1=ALU.mult)

            KT = big.tile([128, KE, T], bf16, name=f"KT{l}", tag="bigA")
            for t in range(KE):
                for jc in range(T // 512):
                    pk = ps1.tile([128, 512], f32, name="pk", tag="ps1")
                    for k in range(KE):
                        mm(pk, w["wk"][:, k, t * 128:(t + 1) * 128],
                           xT_full[:, k, jc * 512:(jc + 1) * 512],
                           start=(k == 0), stop=(k == KE - 1))
                    nc.vector.tensor_scalar_add(
                        KT[:, t, jc * 512:(jc + 1) * 512], pk,
                        w["bk"][:, t, :])

            Vm = big.tile([128, NJB, E], bf16, name=f"Vm{l}", tag="bigB")
            for jb in range(NJB):
                pv = ps1.tile([128, 512], f32, name="pv", tag="ps1")
                mm(pv[:, :E], ones_row, w["bv"], start=True, stop=False,
                   skip_group_check=True)
                for k in range(KE):
                    mm(pv[:, :E], xT_full[:, k, jb * 128:(jb + 1) * 128],
                       w["wv"][:, k, :], start=False, stop=(k == KE - 1),
                       skip_group_check=True)
                nc.vector.tensor_scalar_mul(Vm[:, jb, :], pv[:, :E],
                                            m_tiles[:, jb:jb + 1])

            OTn = work.tile([128, KE, NL], f32r, name=f"OTn{l}", tag="OTn")
            for q in range(2):
                strips = [spool.tile([128, NJB, NL], bf16,
                                     name=f"u{l}{q}{h4}", tag="ustrip",
                                     bufs=4)
                          for h4 in range(4)]
                for jq in range(4):
                    for hp in range(2):
                        scs = [ps2.tile([128, 1024], f32, name="sc",
                                        tag="ps2") for _ in range(2)]
                        for j in range(4):
                            jb = jq * 4 + j
                            for i in range(2):
                                h4 = hp * 2 + i
                                mm(scs[i][:, j * 256:(j + 1) * 256],
                                   KT[h4 * 32:(h4 + 1) * 32, q,
                                      jb * 128:(jb + 1) * 128],
                                   QT[h4 * 32:(h4 + 1) * 32, q, :],
                                   start=True, stop=True,
                                   tile_position=(h4 * 32, 0),
                                   skip_group_check=True)
                        for i in range(2):
                            h4 = hp * 2 + i
                            nc.scalar.activation(
                                out=strips[h4][:, jq * 4:(jq + 1) * 4, :],
                                in_=scs[i], func=AF.Exp)
                OT_ps = ps1.tile([128, 512], f32, name="otps", tag="ps1")
                RS_ps = ps1.tile([128, 512], f32, name="rsps", tag="ps1")
                for jb in range(NJB):
                    for h4 in range(4):
                        h = q * 4 + h4
                        mm(OT_ps[h4 * 32:(h4 + 1) * 32, :NL],
                           Vm[:, jb, h * 32:(h + 1) * 32],
                           strips[h4][:, jb, :],
                           start=(jb == 0 and h4 == 0), stop=(jb == NJB - 1),
                           tile_position=(0, h4 * 32), skip_group_check=True)
                        mm(RS_ps[h4 * 32:(h4 + 1) * 32, :NL],
                           m32[:, jb, :], strips[h4][:, jb, :],
                           start=(jb == 0 and h4 == 0), stop=(jb == NJB - 1),
                           tile_position=(0, h4 * 32), skip_group_check=True)
                rrec = work.tile([128, NL], f32, name="rrec", tag="rrec")
                nc.vector.reciprocal(rrec, RS_ps[:, :NL])
                nc.vector.tensor_mul(OTn[:, q, :], OT_ps[:, :NL], rrec)

            x2 = work.tile([128, NB, E], f32, name=f"x2{l}", tag="x2")
            for b in range(NB):
                po = ps1.tile([128, 512], f32, name="po", tag="ps1")
                mm(po[:, :E], ones_row, w["bo"], start=True, stop=False,
                   skip_group_check=True)
                for q in range(KE):
                    mm(po[:, :E], OTn[:, q, b * 128:(b + 1) * 128],
                       w["wo"][:, q, :], start=False, stop=(q == KE - 1),
                       skip_group_check=True)
                nc.vector.tensor_add(x2[:, b, :], po[:, :E], x_loc[:, b, :])

            def layernorm(dst, src, gname, bname):
                mvb = work.tile([128, NB, 2], f32, name="mvb", tag="ln_mv")
                for b in range(NB):
                    st = work.tile([128, 6], f32, name="st", tag="ln_st")
                    nc.vector.bn_stats(out=st, in_=src[:, b, :])
                    nc.vector.bn_aggr(out=mvb[:, b, :], in_=st)
                # rstd = rsqrt(var+eps) on DVE only (bf16 bit-trick + Newton)
                ve = work.tile([128, NB], f32, name="ve", tag="ln_ve")
                y = work.tile([128, NB], f32, name="lny", tag="ln_y")
                if not LN_NEWTON:
                    sdq = work.tile([128, NB], f32, name="sdq", tag="ln_sd")
                    nc.vector.tensor_scalar_add(ve, mvb[:, :, 1], 1e-5)
                    nc.scalar.activation(out=sdq, in_=ve, func=AF.Sqrt)
                    nc.vector.reciprocal(y, sdq)
                else:
                    nc.vector.tensor_scalar_add(ve, mvb[:, :, 1], 1e-5)
                if LN_NEWTON:
                    vb = work.tile([128, NB], bf16, name="vb", tag="ln_vb")
                    nc.vector.tensor_copy(vb, ve)
                    sh = work.tile([128, NB], mybir.dt.int16, name="sh",
                                   tag="ln_sh")
                    nc.vector.tensor_scalar(sh, vb.bitcast(mybir.dt.int16), 1,
                                            None,
                                            op0=ALU.logical_shift_right)
                    nc.vector.tensor_scalar(sh, sh, -1, 24375, op0=ALU.mult,
                                            op1=ALU.add)
                    tq = work.tile([128, NB], f32, name="lnt", tag="ln_t")
                    nc.vector.tensor_copy(y, sh.bitcast(bf16))
                    for _ in range(3):
                        nc.vector.tensor_mul(tq, y, y)
                        nc.vector.tensor_mul(tq, tq, ve)
                        nc.vector.tensor_scalar(tq, tq, -0.5, 1.5,
                                                op0=ALU.mult, op1=ALU.add)
                        nc.vector.tensor_mul(y, y, tq)
                for b in range(NB):
                    nc.vector.tensor_scalar(dst[:, b, :], src[:, b, :],
                                            mvb[:, b, 0:1], y[:, b:b + 1],
                                            op0=ALU.subtract, op1=ALU.mult)
                    nc.vector.tensor_mul(dst[:, b, :], dst[:, b, :],
                                         lnb[gname])
                    nc.vector.tensor_add(dst[:, b, :], dst[:, b, :],
                                         lnb[bname])

            x_ln = work.tile([128, NB, E], f32, name=f"xln{l}", tag="x_ln")
            layernorm(x_ln, x2, "ln1g", "ln1b")

            x_lnT = work.tile([128, KE, NL], f32r, name=f"xlnT{l}",
                              tag="x_lnT")
            transpose_to(x_lnT, x_ln)
            f1 = big.tile([128, FFD // 128, NL], bf16, name=f"f1{l}",
                          tag="bigB")
            C1 = 0.7978845608028654
            CA = C1 * 0.044715
            for mb in range(FFD // 128):
                pf = ps1.tile([128, 512], f32, name="pf", tag="ps1")
                for k in range(KE):
                    mm(pf[:, :NL], w["w1"][:, k, mb * 128:(mb + 1) * 128],
                       x_lnT[:, k, :], start=(k == 0), stop=(k == KE - 1))
                if not GELU_COMPOSE:
                    nc.scalar.activation(out=f1[:, mb, :], in_=pf[:, :NL],
                                         func=AF.Gelu_apprx_tanh,
                                         bias=w["b1"][:, mb, :], scale=1.0)
                else:
                    # gelu_tanh(h) = 0.5 h (1 + tanh(C1 h + CA h^3))
                    hb = work.tile([128, NL], f32, name="hb", tag="ff_h", bufs=3)
                    ug = work.tile([128, NL], f32, name="ug", tag="ff_u", bufs=3)
                    nc.vector.tensor_scalar_add(hb, pf[:, :NL],
                                                w["b1"][:, mb, :])
                    nc.vector.tensor_mul(ug, hb, hb)
                    nc.vector.tensor_scalar(ug, ug, CA, C1, op0=ALU.mult,
                                            op1=ALU.add)
                    nc.vector.tensor_mul(ug, ug, hb)
                    nc.scalar.activation(out=ug, in_=ug, func=AF.Tanh)
                    nc.vector.tensor_scalar(ug, ug, 1.0, 0.5, op0=ALU.add,
                                            op1=ALU.mult)
                    nc.vector.tensor_mul(f1[:, mb, :], ug, hb)
            x3 = work.tile([128, NB, E], f32, name=f"x3{l}", tag="x3")
            for b in range(NB):
                pf2 = ps1.tile([128, 512], f32, name="pf2", tag="ps1")
                mm(pf2[:, :E], ones_row, w["b2"], start=True, stop=False,
                   skip_group_check=True)
                for km in range(FFD // 128):
                    mm(pf2[:, :E], f1[:, km, b * 128:(b + 1) * 128],
                       w["w2"][:, km, :], start=False,
                       stop=(km == FFD // 128 - 1), skip_group_check=True)
                nc.vector.tensor_add(x3[:, b, :], pf2[:, :E], x_ln[:, b, :])

            x_new = state.tile([128, NB, E], f32, name=f"xn{l}", tag="h_loc")
            layernorm(x_new, x3, "ln2g", "ln2b")

            x_loc = x_new
            x_locT = state.tile([128, KE, NL], f32r, name=f"xnT{l}",
                                tag="h_locT")
            transpose_to(x_locT, x_loc)
            if l < n_tb - 1:
                xT_full = big.tile([128, KE, T], f32r, name=f"xTf{l}",
                                   tag="hT_full", bufs=2)
                allgather(x_locT, xT_full)

        nc.sync.dma_start(out=out_x.rearrange("b p e -> p b e"), in_=x_loc)


# ---------------- host side ----------------

def _t2(a):
    x = a.shape[0] // 128
    a2 = a.reshape(x, 128, -1).transpose(1, 0, 2)
    return np.ascontiguousarray(a2).astype(np.float32)


def _prep_inputs(input_node, inputad, res, inputtext, linenode, modification,
                 churn, params, n_tb=NTB, n_gg=2):
    f = np.float32
    tok = np.asarray(params["tok_emb"], f)
    tok1 = np.asarray(params["tok_emb1"], f)
    inode = np.asarray(input_node)
    lnode = np.asarray(linenode)
    nodeem = tok[inode[0]]
    x_node = np.concatenate([nodeem, np.asarray(inputtext, f)[0][:, None]], 1)
    lineem = tok1[lnode[0]]
    x_line = np.concatenate(
        [lineem, np.asarray(modification, f)[0][:, None],
         np.asarray(churn, f)[0][:, None]], 1)
    h0 = np.concatenate([x_node, x_line], 0).astype(f)

    mask = np.concatenate([(inode[0] > 0), np.ones(NLLEN, bool)]).astype(f)
    m_tiles = np.ascontiguousarray(mask.reshape(NJB, 128).T).astype(f)
    m32 = np.repeat(m_tiles[:, :, None], 32, axis=2).astype(ml_dtypes.bfloat16)

    adj = np.asarray(inputad, f)

    com = {
        "h0T_full": _t2(np.ascontiguousarray(h0.T)),
        "m_tiles": m_tiles,
        "m32": m32,
        "ident": np.eye(128, dtype=f),
    }
    for g, key in enumerate(["g1", "g2"][:n_gg]):
        gp = params[key]
        W = np.asarray(gp["W"], f)
        a = np.asarray(gp["a"], f)
        gr = gp["gru"]
        Wi = np.asarray(gr["Wi"], f)
        Wh = np.asarray(gr["Wh"], f)
        com[f"g{g}_Wx"] = _t2(np.concatenate([W, W @ a[E:]], 1))
        com[f"g{g}_Wa1"] = _t2(W @ a[:E])
        com[f"g{g}_WiT"] = _t2(np.ascontiguousarray(Wi.T))
        com[f"g{g}_WhT"] = _t2(np.ascontiguousarray(Wh.T))
        com[f"g{g}_bi"] = np.asarray(gr["bi"], f).reshape(1, -1)
        com[f"g{g}_bh"] = np.asarray(gr["bh"], f).reshape(1, -1)
    for l in range(n_tb):
        tb = params["tblocks"][l]
        gv = lambda k: np.asarray(tb[k], f)
        com[f"t{l}_wq"] = _t2(gv("Wq"))
        com[f"t{l}_wk"] = _t2(gv("Wk"))
        com[f"t{l}_wv"] = _t2(gv("Wv"))
        com[f"t{l}_wo"] = _t2(gv("Wo"))
        com[f"t{l}_bq"] = _t2(gv("bq").reshape(E, 1))
        com[f"t{l}_bk"] = _t2(gv("bk").reshape(E, 1))
        com[f"t{l}_bv"] = gv("bv").reshape(1, E)
        com[f"t{l}_bo"] = gv("bo").reshape(1, E)
        com[f"t{l}_w1"] = _t2(gv("W1"))
        com[f"t{l}_b1"] = _t2(gv("b1").reshape(FFD, 1))
        com[f"t{l}_w2"] = _t2(gv("W2")).astype(ml_dtypes.bfloat16)
        com[f"t{l}_b2"] = gv("b2").reshape(1, E)
        for nm, key in [("ln1g", "ln1_g"), ("ln1b", "ln1_b"),
                        ("ln2g", "ln2_g"), ("ln2b", "ln2_b")]:
            com[f"t{l}_{nm}"] = gv(key).reshape(1, E)

    bf_keys = ["h0T_full"]
    for g in range(n_gg):
        bf_keys += [f"g{g}_{s}" for s in
                    ["Wx", "Wa1", "WiT", "WhT", "bi", "bh"]]
    for l in range(n_tb):
        bf_keys += [f"t{l}_{s}" for s in
                    ["wq", "wk", "wv", "wo", "bv", "bo", "w1", "b2"]]
    for k in bf_keys:
        com[k] = com[k].astype(ml_dtypes.bfloat16)

    in_maps = []
    for r in range(R):
        blk = h0[r * NL:(r + 1) * NL]
        d = dict(com)
        d["h0_loc"] = _t2(blk)
        d["h0_locT"] = _t2(np.ascontiguousarray(blk.T)).astype(
            ml_dtypes.bfloat16)
        d["adjT"] = _t2(np.ascontiguousarray(
            adj[r * NL:(r + 1) * NL].T)).astype(ml_dtypes.bfloat16)
        in_maps.append(d)
    return in_maps, h0


_CACHE = {}


def run_device(in_maps, n_tb=NTB, n_gg=2, trace=False):
    key = (n_tb, n_gg)
    if key not in _CACHE:
        _CACHE[key] = build_bass(n_tb, n_gg)
    nc = _CACHE[key]
    return run_bass_kernel_spmd(nc, in_maps, core_ids=list(range(R)),
                                trace=trace)


def kernel(input_node, inputad, res, inputtext, linenode, modification, churn,
           params, _trace=False, _n_tb=NTB, _n_gg=2):
    in_maps, _ = _prep_inputs(input_node, inputad, res, inputtext, linenode,
                              modification, churn, params, _n_tb, _n_gg)
    out = run_device(in_maps, _n_tb, _n_gg, trace=_trace)
    blocks = [np.asarray(out.results[r]["out_x"]).reshape(NL, E)
              for r in range(R)]
    x_full = np.concatenate(blocks, 0)
    kernel._x_full = x_full
    kernel._profile = out

    f = np.float32
    x = x_full[:NLLEN][None]
    wv = np.asarray(params["res2_w"], f)
    bv = np.asarray(params["res2_b"], f)
    logits = (x @ wv + bv).squeeze(-1).astype(f)
    inode = np.asarray(input_node)
    resmask = inode == 2
    logits = np.where(resmask, logits, f(-1e9)).astype(f)
    zmax = logits.max(-1, keepdims=True)
    ez = np.exp(logits - zmax, dtype=f)
    psm = (ez / ez.sum(-1, keepdims=True)).astype(f)
    loss = np.sum(-np.log(np.clip(psm, 1e-10, 1.0)) * np.asarray(res, f),
                  axis=-1).astype(f)
    return loss, psm, x.astype(f)


# revision 17
# speedup vs baseline: 4303.6610x; 1.0002x over previous
"""Trainium2 Bass kernel for nn_NlEncoder (GNN message passing + transformer).

Sharding: N=2048 rows split across 8 cores (256 rows each). Each layer: every
core computes its row-block of scores/attention against all-gathered key-side
tensors, then local GRU/FFN. AllGather of transposed local blocks between
layers. Embedding gather + tiny output head run on host.

Device layouts per core:
  natural    [128, NB, X]: row n = b*128+p on partitions, features free
  transposed [128, KE, N]: feature e = k*128+p on partitions, rows free
All matmuls run with bf16 operands (full-rate PE + fast weight load) and
f32 PSUM accumulation; norms, GRU state and residuals stay in f32.
"""

import contextlib
import numpy as np
import ml_dtypes

import concourse.bass as bass
import concourse.bacc as bacc
import concourse.tile as tile
from concourse import mybir
from concourse.bass_utils import run_bass_kernel_spmd

R = 8
T = 2048
NL = T // R          # 256
E = 256
H = 8
D = 32
FFD = 1024
NTB = 5
NJB = T // 128       # 16
KE = E // 128        # 2
NB = NL // 128       # 2
ALPHA = 0.2
NLLEN = 1024
GELU_COMPOSE = False
LN_NEWTON = False

f32 = mybir.dt.float32
f32r = mybir.dt.bfloat16  # matmul operand dtype (PE full rate, FWL)
bf16 = mybir.dt.bfloat16
AF = mybir.ActivationFunctionType
ALU = mybir.AluOpType


def r32(ap):
    assert ap.dtype != f32, f"f32 operand reached matmul: {ap}"
    return ap


def build_bass(n_tb=NTB, n_gg=2):
    # variant flags: GELU_COMPOSE/LN_NEWTON module-level
    nc = bacc.Bacc("TRN2", num_devices=R, name="nlenc")
    with tile.TileContext(nc) as tc:
        _build_body(nc, tc, n_tb, n_gg)
    nc.compile()
    return nc


def _build_body(nc, tc, n_tb, n_gg):
    din = {}

    def inp(name, shape, dtype=f32):
        din[name] = nc.dram_tensor(name, list(shape), dtype,
                                   kind="ExternalInput")
        return din[name]

    def dap(name):
        h = din[name]
        return h[tuple(slice(None) for _ in h.shape)]

    inp("h0T_full", [128, KE, T], f32r)
    inp("h0_loc", [128, NB, E])
    inp("h0_locT", [128, KE, NL], f32r)
    inp("adjT", [128, NJB, NL], bf16)
    inp("m_tiles", [128, NJB])
    inp("m32", [128, NJB, 32], bf16)
    inp("ident", [128, 128])
    for g in range(n_gg):
        inp(f"g{g}_Wx", [128, KE, E + 1], f32r)
        inp(f"g{g}_Wa1", [128, KE, 1], f32r)
        inp(f"g{g}_WiT", [128, KE, 3 * E], f32r)
        inp(f"g{g}_WhT", [128, KE, 3 * E], f32r)
        inp(f"g{g}_bi", [1, 3 * E], f32r)
        inp(f"g{g}_bh", [1, 3 * E], f32r)
    TBW = [("wq", [128, KE, E], f32r), ("wk", [128, KE, E], f32r),
           ("wv", [128, KE, E], f32r), ("wo", [128, KE, E], f32r),
           ("bq", [128, KE, 1], f32), ("bk", [128, KE, 1], f32),
           ("bv", [1, E], f32r), ("bo", [1, E], f32r),
           ("w1", [128, KE, FFD], f32r), ("b1", [128, FFD // 128, 1], f32),
           ("w2", [128, FFD // 128, E], bf16), ("b2", [1, E], f32r),
           ("ln1g", [1, E], f32), ("ln1b", [1, E], f32),
           ("ln2g", [1, E], f32), ("ln2b", [1, E], f32)]
    for l in range(n_tb):
        for nm, shp, dt_ in TBW:
            inp(f"t{l}_{nm}", shp, dt_)

    out_x = nc.dram_tensor("out_x", [NB, 128, E], f32, kind="ExternalOutput")

    ctx = contextlib.ExitStack()
    with ctx:
        sing = ctx.enter_context(tc.tile_pool(name="sing", bufs=1))
        state = ctx.enter_context(tc.tile_pool(name="state", bufs=2))
        big = ctx.enter_context(tc.tile_pool(name="bigsb", bufs=1))
        wpool = ctx.enter_context(tc.tile_pool(name="wpool", bufs=1))
        work = ctx.enter_context(tc.tile_pool(name="work", bufs=1))
        spool = ctx.enter_context(tc.tile_pool(name="spool", bufs=2))
        ps1 = ctx.enter_context(tc.tile_pool(name="ps1", bufs=4, space="PSUM"))
        ps2 = ctx.enter_context(tc.tile_pool(name="ps2", bufs=2, space="PSUM"))
        dram = ctx.enter_context(tc.tile_pool(name="dram", bufs=2,
                                              space="DRAM"))

        def mm(out, lhsT, rhs, **kw):
            nc.tensor.matmul(out, r32(lhsT), r32(rhs), **kw)

        # ---- constants ----
        ident = sing.tile([128, 128], f32, name="identc")
        nc.sync.dma_start(out=ident, in_=dap("ident"))
        ones_row = sing.tile([1, 128], f32r, name="ones_row")
        nc.vector.memset(ones_row, 1.0)
        eps_t = sing.tile([128, 1], f32, name="eps_t")
        nc.vector.memset(eps_t, 1e-5)
        adjT = sing.tile([128, NJB, NL], bf16, name="adjTc")
        nc.sync.dma_start(out=adjT, in_=dap("adjT"))
        m_tiles = sing.tile([128, NJB], f32, name="m_tilesc")
        nc.sync.dma_start(out=m_tiles, in_=dap("m_tiles"))
        m32 = sing.tile([128, NJB, 32], bf16, name="m32c")
        nc.sync.dma_start(out=m32, in_=dap("m32"))

        h_loc = state.tile([128, NB, E], f32, name="h_loc", tag="h_loc")
        h_locT = state.tile([128, KE, NL], f32r, name="h_locT", tag="h_locT")
        hT_full = big.tile([128, KE, T], f32r, name="hT_full", tag="hT_full",
                           bufs=2)
        nc.sync.dma_start(out=h_loc, in_=dap("h0_loc"))
        nc.sync.dma_start(out=h_locT, in_=dap("h0_locT"))
        for k in range(KE):
            for jh in range(2):
                nc.sync.dma_start(
                    out=hT_full[:, k, jh * (T // 2):(jh + 1) * (T // 2)],
                    in_=dap("h0T_full")[:, k,
                                        jh * (T // 2):(jh + 1) * (T // 2)])

        def cp(i, out, in_):
            if i % 2:
                nc.scalar.copy(out=out, in_=in_)
            else:
                nc.vector.tensor_copy(out=out, in_=in_)

        def transpose_to(dst, src_nat):
            for b in range(NB):
                for k in range(KE):
                    tp = ps1.tile([128, 512], f32, name="tp", tag="ps1")
                    nc.tensor.transpose(
                        tp[:, :128], src_nat[:, b, k * 128:(k + 1) * 128],
                        ident)
                    cp(b * KE + k, dst[:, k, b * 128:(b + 1) * 128],
                       tp[:, :128])

        def allgather(srcT, dstT_full):
            cont = dram.tile([KE, 128, NL], f32r, name="agin", tag="agin")
            gout = dram.tile([R, KE, 128, NL], f32r, name="agout", tag="agout",
                             addr_space="Shared")
            nc.sync.dma_start(out=cont.rearrange("k p n -> p k n"), in_=srcT)
            nc.gpsimd.collective_compute(
                "AllGather", ALU.bypass,
                replica_groups=[list(range(R))],
                ins=[cont.opt()], outs=[gout.opt()],
            )
            for k in range(KE):
                for rh in range(4):
                    nc.sync.dma_start(
                        out=dstT_full[:, k, rh * (T // 4):(rh + 1) * (T // 4)]
                            .rearrange("p (r n) -> p r n", r=R // 4),
                        in_=gout[rh * (R // 4):(rh + 1) * (R // 4), k, :, :]
                            .rearrange("r p n -> p r n"),
                    )

        # ================= GGANN =================
        for g in range(n_gg):
            Wx = wpool.tile([128, KE, E + 1], f32r, name=f"Wx{g}", tag="wsmall",
                            bufs=8)
            Wa1 = wpool.tile([128, KE, 1], f32r, name=f"Wa1{g}", tag="wcol",
                             bufs=8)
            WiT = wpool.tile([128, KE, 3 * E], f32r, name=f"WiT{g}", tag="wbig",
                             bufs=3)
            WhT = wpool.tile([128, KE, 3 * E], f32r, name=f"WhT{g}", tag="wbig",
                             bufs=3)
            bi_r = wpool.tile([1, 3 * E], f32r, name=f"bi{g}", tag="wvec",
                              bufs=4)
            bh_r = wpool.tile([1, 3 * E], f32r, name=f"bh{g}", tag="wvec",
                              bufs=4)
            for nm, tl in [("Wx", Wx), ("Wa1", Wa1), ("WiT", WiT),
                           ("WhT", WhT), ("bi", bi_r), ("bh", bh_r)]:
                nc.sync.dma_start(out=tl, in_=dap(f"g{g}_{nm}"))

            # Wh(+e2) per j-block; Whg bf16 with trailing ones column
            Whg = big.tile([128, NJB, E + 1], bf16, name=f"Whg{g}", tag="bigA")
            e2c = work.tile([128, NJB], f32, name=f"e2c{g}", tag="e2c")
            for jb in range(NJB):
                pw = ps1.tile([128, 512], f32, name="pw", tag="ps1")
                for k in range(KE):
                    mm(pw[:, :E + 1], hT_full[:, k, jb * 128:(jb + 1) * 128],
                       Wx[:, k, :], start=(k == 0), stop=(k == KE - 1))
                cp(jb, Whg[:, jb, :E], pw[:, :E])
                nc.vector.tensor_copy(e2c[:, jb:jb + 1], pw[:, E:E + 1])
                nc.vector.memset(Whg[:, jb, E:E + 1], 1.0)

            pe1 = ps1.tile([128, 512], f32, name="pe1", tag="ps1")
            for k in range(KE):
                mm(pe1[:1, :NL], Wa1[:, k, :], h_locT[:, k, :],
                   start=(k == 0), stop=(k == KE - 1))
            e1row = work.tile([1, NL], f32r, name="e1row", tag="e1row")
            nc.vector.tensor_copy(e1row, pe1[:1, :NL])
            pb = ps1.tile([128, 512], f32, name="pb", tag="ps1")
            mm(pb[:, :NL], ones_row, e1row, start=True, stop=True)
            E1b = work.tile([128, NL], f32, name="E1b", tag="E1b")
            nc.vector.tensor_copy(E1b, pb[:, :NL])

            hp_ps = [ps1.tile([128, 512], f32, name=f"hp{b}", tag="ps1")
                     for b in range(NB)]
            for half in range(2):
                strip = spool.tile([128, NJB // 2, NL], bf16,
                                   name=f"str{g}{half}", tag="ggstrip")
                for j in range(NJB // 2):
                    jb = half * (NJB // 2) + j
                    nc.scalar.activation(
                        out=strip[:, j, :], in_=E1b, func=AF.Prelu,
                        bias=e2c[:, jb:jb + 1], scale=1.0, alpha=ALPHA)
                nc.scalar.activation(out=strip[:, :, :], in_=strip[:, :, :],
                                     func=AF.Exp)
                nc.vector.tensor_mul(
                    strip[:, :, :], strip[:, :, :],
                    adjT[:, half * (NJB // 2):(half + 1) * (NJB // 2), :])
                for j in range(NJB // 2):
                    jb = half * (NJB // 2) + j
                    for b in range(NB):
                        mm(hp_ps[b][:, :E + 1],
                           strip[:, j, b * 128:(b + 1) * 128],
                           Whg[:, jb, :],
                           start=(jb == 0), stop=(jb == NJB - 1),
                           skip_group_check=True)

            h_p = work.tile([128, NB, E], f32, name=f"h_p{g}", tag="h_p")
            for b in range(NB):
                rsum = work.tile([128, 1], f32, name=f"rs{g}{b}", tag="rsum")
                nc.vector.tensor_scalar_add(rsum, hp_ps[b][:, E:E + 1], 1e-30)
                rrec = work.tile([128, 1], f32, name=f"rr{g}{b}", tag="rrec")
                nc.vector.reciprocal(rrec, rsum)
                nc.vector.tensor_scalar_mul(h_p[:, b, :], hp_ps[b][:, :E],
                                            rrec)

            h_pT = work.tile([128, KE, NL], f32r, name=f"h_pT{g}", tag="h_pT")
            transpose_to(h_pT, h_p)
            h_new = state.tile([128, NB, E], f32, name=f"h_new{g}",
                               tag="h_loc")
            for b in range(NB):
                # r/z gates: gi+gh accumulated jointly in one PSUM tile
                prz = ps1.tile([128, 512], f32, name="prz", tag="ps1")
                mm(prz, ones_row, bi_r[:, 0:512], start=True, stop=False,
                   skip_group_check=True)
                mm(prz, ones_row, bh_r[:, 0:512], start=False, stop=False,
                   skip_group_check=True)
                for k in range(KE):
                    mm(prz, h_pT[:, k, b * 128:(b + 1) * 128],
                       WiT[:, k, 0:512], start=False, stop=False,
                       skip_group_check=True)
                    mm(prz, h_locT[:, k, b * 128:(b + 1) * 128],
                       WhT[:, k, 0:512], start=False, stop=(k == KE - 1),
                       skip_group_check=True)
                # n gate: inn and hn separate
                pin = ps1.tile([128, 512], f32, name="pin", tag="ps1")
                phn = ps1.tile([128, 512], f32, name="phn", tag="ps1")
                mm(pin[:, :E], ones_row, bi_r[:, 512:768], start=True,
                   stop=False, skip_group_check=True)
                mm(phn[:, :E], ones_row, bh_r[:, 512:768], start=True,
                   stop=False, skip_group_check=True)
                for k in range(KE):
                    mm(pin[:, :E], h_pT[:, k, b * 128:(b + 1) * 128],
                       WiT[:, k, 512:768], start=False, stop=(k == KE - 1),
                       skip_group_check=True)
                    mm(phn[:, :E], h_locT[:, k, b * 128:(b + 1) * 128],
                       WhT[:, k, 512:768], start=False, stop=(k == KE - 1),
                       skip_group_check=True)
                trz = work.tile([128, 2 * E], f32, name="trz", tag="gr_rz")
                tn = work.tile([128, E], f32, name="tn", tag="gr_n")
                t2 = work.tile([128, E], f32, name="t2", tag="gr_t")
                # sigmoid(x) = 1/(1+exp(-x)) -- keeps ACT in the exp table set
                nc.scalar.activation(out=trz, in_=prz[:, 0:2 * E],
                                     func=AF.Exp, scale=-1.0)
                nc.vector.tensor_scalar_add(trz, trz, 1.0)
                nc.vector.reciprocal(trz, trz)
                tr = trz[:, 0:E]
                tz = trz[:, E:2 * E]
                nc.vector.tensor_mul(tn, tr, phn[:, :E])
                nc.vector.tensor_add(tn, tn, pin[:, :E])
                nc.scalar.activation(out=tn, in_=tn, func=AF.Tanh)
                nc.vector.tensor_sub(t2, h_loc[:, b, :], tn)
                nc.vector.tensor_mul(t2, t2, tz)
                nc.vector.tensor_add(h_new[:, b, :], t2, tn)

            if g == 0:
                ex = work.tile([128, NB, E], f32, name="elu_e", tag="elu_e")
                mk = work.tile([128, NB, E], f32, name="elu_m", tag="elu_m")
                nc.scalar.activation(out=ex[:, :, :], in_=h_new[:, :, :],
                                     func=AF.Exp)
                nc.vector.tensor_scalar(mk[:, :, :], h_new[:, :, :], 0.0,
                                        None, op0=ALU.is_gt)
                nc.vector.tensor_scalar_add(ex[:, :, :], ex[:, :, :], -1.0)
                nc.vector.tensor_sub(h_new[:, :, :], h_new[:, :, :],
                                     ex[:, :, :])
                nc.vector.tensor_mul(h_new[:, :, :], h_new[:, :, :],
                                     mk[:, :, :])
                nc.vector.tensor_add(h_new[:, :, :], h_new[:, :, :],
                                     ex[:, :, :])

            h_loc = h_new
            h_locT = state.tile([128, KE, NL], f32r, name=f"h_nT{g}",
                                tag="h_locT")
            transpose_to(h_locT, h_loc)
            if g < n_gg - 1 or n_tb > 0:
                hT_full = big.tile([128, KE, T], f32r, name=f"hTf{g}",
                                   tag="hT_full", bufs=2)
                allgather(h_locT, hT_full)

        # ================= transformer =================
        x_loc, x_locT, xT_full = h_loc, h_locT, hT_full
        for l in range(n_tb):
            w = {}
            for nm, shp, dt_ in TBW:
                if nm in ("wq", "wk", "wv", "wo"):
                    tg, bf = "wsmall", 8
                elif nm in ("w1", "w2"):
                    tg, bf = "wbig", 3
                elif nm in ("bv", "bo", "b2"):
                    tg, bf = "wvec", 4
                elif nm in ("bq", "bk", "b1"):
                    tg, bf = "wcol", 8
                else:
                    tg, bf = None, None
                if tg is not None:
                    w[nm] = wpool.tile(shp, dt_, name=f"{nm}{l}", tag=tg,
                                       bufs=bf)
                    nc.sync.dma_start(out=w[nm], in_=dap(f"t{l}_{nm}"))
            lnb = {}
            for nm in ["ln1g", "ln1b", "ln2g", "ln2b"]:
                t = wpool.tile([128, E], f32, name=f"{nm}b{l}", tag="lnb",
                               bufs=8)
                src = dap(f"t{l}_{nm}")
                nc.sync.dma_start(
                    out=t,
                    in_=bass.AP(tensor=src.tensor, offset=src.offset,
                                ap=[[0, 128], src.ap[-1]]))
                lnb[nm] = t

            QT = work.tile([128, KE, NL], bf16, name=f"QT{l}", tag="QT")
            for t in range(KE):
                pq = ps1.tile([128, 512], f32, name="pq", tag="ps1")
                for k in range(KE):
                    mm(pq[:, :NL], w["wq"][:, k, t * 128:(t + 1) * 128],
                       x_locT[:, k, :], start=(k == 0), stop=(k == KE - 1))
                nc.vector.tensor_scalar(
                    QT[:, t, :], pq[:, :NL], w["bq"][:, t, :],
                    float(1.0 / np.sqrt(D)), op0=ALU.add, op1=ALU.mult)

            KT = big.tile([128, KE, T], bf16, name=f"KT{l}", tag="bigA")
            for t in range(KE):
                for jc in range(T // 512):
                    pk = ps1.tile([128, 512], f32, name="pk", tag="ps1")
                    for k in range(KE):
                        mm(pk, w["wk"][:, k, t * 128:(t + 1) * 128],
                           xT_full[:, k, jc * 512:(jc + 1) * 512],
                           start=(k == 0), stop=(k == KE - 1))
                    nc.vector.tensor_scalar_add(
                        KT[:, t, jc * 512:(jc + 1) * 512], pk,
                        w["bk"][:, t, :])

            Vm = big.tile([128, NJB, E], bf16, name=f"Vm{l}", tag="bigB")
            for jb in range(NJB):
                pv = ps1.tile([128, 512], f32, name="pv", tag="ps1")
                mm(pv[:, :E], ones_row, w["bv"], start=True, stop=False,
                   skip_group_check=True)
                for k in range(KE):
                    mm(pv[:, :E], xT_full[:, k, jb * 128:(jb + 1) * 128],
                       w["wv"][:, k, :], start=False, stop=(k == KE - 1),
                       skip_group_check=True)
                nc.vector.tensor_scalar_mul(Vm[:, jb, :], pv[:, :E],
                                            m_tiles[:, jb:jb + 1])

            OTn = work.tile([128, KE, NL], f32r, name=f"OTn{l}", tag="OTn")
            for q in range(2):
                strips = [spool.tile([128, NJB, NL], bf16,
                                     name=f"u{l}{q}{h4}", tag="ustrip",
                                     bufs=4)
                          for h4 in range(4)]
                for jq in range(4):
                    for hp in range(2):
                        scs = [ps2.tile([128, 1024], f32, name="sc",
                                        tag="ps2") for _ in range(2)]
                        for j in range(4):
                            jb = jq * 4 + j
                            for i in range(2):
                                h4 = hp * 2 + i
                                mm(scs[i][:, j * 256:(j + 1) * 256],
                                   KT[h4 * 32:(h4 + 1) * 32, q,
                                      jb * 128:(jb + 1) * 128],
                                   QT[h4 * 32:(h4 + 1) * 32, q, :],
                                   start=True, stop=True,
                                   tile_position=(h4 * 32, 0),
                                   skip_group_check=True)
                        for i in range(2):
                            h4 = hp * 2 + i
                            nc.scalar.activation(
                                out=strips[h4][:, jq * 4:(jq + 1) * 4, :],
                                in_=scs[i], func=AF.Exp)
                OT_ps = ps1.tile([128, 512], f32, name="otps", tag="ps1")
                RS_ps = ps1.tile([128, 512], f32, name="rsps", tag="ps1")
                for jb in range(NJB):
                    for h4 in range(4):
                        h = q * 4 + h4
                        mm(OT_ps[h4 * 32:(h4 + 1) * 32, :NL],
                           Vm[:, jb, h * 32:(h + 1) * 32],
                           strips[h4][:, jb, :],
                           start=(jb == 0 and h4 == 0), stop=(jb == NJB - 1),
                           tile_position=(0, h4 * 32), skip_group_check=True)
                        mm(RS_ps[h4 * 32:(h4 + 1) * 32, :NL],
                           m32[:, jb, :], strips[h4][:, jb, :],
                           start=(jb == 0 and h4 == 0), stop=(jb == NJB - 1),
                           tile_position=(0, h4 * 32), skip_group_check=True)
                rrec = work.tile([128, NL], f32, name="rrec", tag="rrec")
                nc.vector.reciprocal(rrec, RS_ps[:, :NL])
                nc.vector.tensor_mul(OTn[:, q, :], OT_ps[:, :NL], rrec)

            x2 = work.tile([128, NB, E], f32, name=f"x2{l}", tag="x2")
            for b in range(NB):
                po = ps1.tile([128, 512], f32, name="po", tag="ps1")
                mm(po[:, :E], ones_row, w["bo"], start=True, stop=False,
                   skip_group_check=True)
                for q in range(KE):
                    mm(po[:, :E], OTn[:, q, b * 128:(b + 1) * 128],
                       w["wo"][:, q, :], start=False, stop=(q == KE - 1),
                       skip_group_check=True)
                nc.vector.tensor_add(x2[:, b, :], po[:, :E], x_loc[:, b, :])

            def layernorm(dst, src, gname, bname):
                mvb = work.tile([128, NB, 2], f32, name="mvb", tag="ln_mv")
                for b in range(NB):
                    st = work.tile([128, 6], f32, name="st", tag="ln_st")
                    nc.vector.bn_stats(out=st, in_=src[:, b, :])
                    nc.vector.bn_aggr(out=mvb[:, b, :], in_=st)
                # rstd = rsqrt(var+eps) on DVE only (bf16 bit-trick + Newton)
                ve = work.tile([128, NB], f32, name="ve", tag="ln_ve")
                y = work.tile([128, NB], f32, name="lny", tag="ln_y")
                if not LN_NEWTON:
                    sdq = work.tile([128, NB], f32, name="sdq", tag="ln_sd")
                    nc.vector.tensor_scalar_add(ve, mvb[:, :, 1], 1e-5)
                    nc.scalar.activation(out=sdq, in_=ve, func=AF.Sqrt)
                    nc.vector.reciprocal(y, sdq)
                else:
                    nc.vector.tensor_scalar_add(ve, mvb[:, :, 1], 1e-5)
                if LN_NEWTON:
                    vb = work.tile([128, NB], bf16, name="vb", tag="ln_vb")
                    nc.vector.tensor_copy(vb, ve)
                    sh = work.tile([128, NB], mybir.dt.int16, name="sh",
                                   tag="ln_sh")
                    nc.vector.tensor_scalar(sh, vb.bitcast(mybir.dt.int16), 1,
                                            None,
                                            op0=ALU.logical_shift_right)
                    nc.vector.tensor_scalar(sh, sh, -1, 24375, op0=ALU.mult,
                                            op1=ALU.add)
                    tq = work.tile([128, NB], f32, name="lnt", tag="ln_t")
                    nc.vector.tensor_copy(y, sh.bitcast(bf16))
                    for _ in range(3):
                        nc.vector.tensor_mul(tq, y, y)
                        nc.vector.tensor_mul(tq, tq, ve)
                        nc.vector.tensor_scalar(tq, tq, -0.5, 1.5,
                                                op0=ALU.mult, op1=ALU.add)
                        nc.vector.tensor_mul(y, y, tq)
                for b in range(NB):
                    nc.vector.tensor_scalar(dst[:, b, :], src[:, b, :],
                                            mvb[:, b, 0:1], y[:, b:b + 1],
                                            op0=ALU.subtract, op1=ALU.mult)
                    nc.vector.tensor_mul(dst[:, b, :], dst[:, b, :],
                                         lnb[gname])
                    nc.vector.tensor_add(dst[:, b, :], dst[:, b, :],
                                         lnb[bname])

            x_ln = work.tile([128, NB, E], f32, name=f"xln{l}", tag="x_ln")
            layernorm(x_ln, x2, "ln1g", "ln1b")

            x_lnT = work.tile([128, KE, NL], f32r, name=f"xlnT{l}",
                              tag="x_lnT")
            transpose_to(x_lnT, x_ln)
            f1 = big.tile([128, FFD // 128, NL], bf16, name=f"f1{l}",
                          tag="bigB")
            C1 = 0.7978845608028654
            CA = C1 * 0.044715
            for mb in range(FFD // 128):
                pf = ps1.tile([128, 512], f32, name="pf", tag="ps1")
                for k in range(KE):
                    mm(pf[:, :NL], w["w1"][:, k, mb * 128:(mb + 1) * 128],
                       x_lnT[:, k, :], start=(k == 0), stop=(k == KE - 1))
                if not GELU_COMPOSE:
                    nc.scalar.activation(out=f1[:, mb, :], in_=pf[:, :NL],
                                         func=AF.Gelu_apprx_tanh,
                                         bias=w["b1"][:, mb, :], scale=1.0)
                else:
                    # gelu_tanh(h) = 0.5 h (1 + tanh(C1 h + CA h^3))
                    hb = work.tile([128, NL], f32, name="hb", tag="ff_h", bufs=3)
                    ug = work.tile([128, NL], f32, name="ug", tag="ff_u", bufs=3)
                    nc.vector.tensor_scalar_add(hb, pf[:, :NL],
                                                w["b1"][:, mb, :])
                    nc.vector.tensor_mul(ug, hb, hb)
                    nc.vector.tensor_scalar(ug, ug, CA, C1, op0=ALU.mult,
                                            op1=ALU.add)
                    nc.vector.tensor_mul(ug, ug, hb)
                    nc.scalar.activation(out=ug, in_=ug, func=AF.Tanh)
                    nc.vector.tensor_scalar(ug, ug, 1.0, 0.5, op0=ALU.add,
                                            op1=ALU.mult)
                    nc.vector.tensor_mul(f1[:, mb, :], ug, hb)
            x3 = work.tile([128, NB, E], f32, name=f"x3{l}", tag="x3")
            for b in range(NB):
                pf2 = ps1.tile([128, 512], f32, name="pf2", tag="ps1")
                mm(pf2[:, :E], ones_row, w["b2"], start=True, stop=False,
                   skip_group_check=True)
                for km in range(FFD // 128):
                    mm(pf2[:, :E], f1[:, km, b * 128:(b + 1) * 128],
                       w["w2"][:, km, :], start=False,
                       stop=(km == FFD // 128 - 1), skip_group_check=True)
                nc.vector.tensor_add(x3[:, b, :], pf2[:, :E], x_ln[:, b, :])

            x_new = state.tile([128, NB, E], f32, name=f"xn{l}", tag="h_loc")
            layernorm(x_new, x3, "ln2g", "ln2b")

            x_loc = x_new
            x_locT = state.tile([128, KE, NL], f32r, name=f"xnT{l}",
                                tag="h_locT")
            transpose_to(x_locT, x_loc)
            if l < n_tb - 1:
                xT_full = big.tile([128, KE, T], f32r, name=f"xTf{l}",
                                   tag="hT_full", bufs=2)
                allgather(x_locT, xT_full)

        nc.sync.dma_start(out=out_x.rearrange("b p e -> p b e"), in_=x_loc)


# ---------------- host side ----------------

def _t2(a):
    x = a.shape[0] // 128
    a2 = a.reshape(x, 128, -1).transpose(1, 0, 2)
    return np.ascontiguousarray(a2).astype(np.float32)


def _prep_inputs(input_node, inputad, res, inputtext, linenode, modification,
                 churn, params, n_tb=NTB, n_gg=2):
    f = np.float32
    tok = np.asarray(params["tok_emb"], f)
    tok1 = np.asarray(params["tok_emb1"], f)
    inode = np.asarray(input_node)
    lnode = np.asarray(linenode)
    nodeem = tok[inode[0]]
    x_node = np.concatenate([nodeem, np.asarray(inputtext, f)[0][:, None]], 1)
    lineem = tok1[lnode[0]]
    x_line = np.concatenate(
        [lineem, np.asarray(modification, f)[0][:, None],
         np.asarray(churn, f)[0][:, None]], 1)
    h0 = np.concatenate([x_node, x_line], 0).astype(f)

    mask = np.concatenate([(inode[0] > 0), np.ones(NLLEN, bool)]).astype(f)
    m_tiles = np.ascontiguousarray(mask.reshape(NJB, 128).T).astype(f)
    m32 = np.repeat(m_tiles[:, :, None], 32, axis=2).astype(ml_dtypes.bfloat16)

    adj = np.asarray(inputad, f)

    com = {
        "h0T_full": _t2(np.ascontiguousarray(h0.T)),
        "m_tiles": m_tiles,
        "m32": m32,
        "ident": np.eye(128, dtype=f),
    }
    for g, key in enumerate(["g1", "g2"][:n_gg]):
        gp = params[key]
        W = np.asarray(gp["W"], f)
        a = np.asarray(gp["a"], f)
        gr = gp["gru"]
        Wi = np.asarray(gr["Wi"], f)
        Wh = np.asarray(gr["Wh"], f)
        com[f"g{g}_Wx"] = _t2(np.concatenate([W, W @ a[E:]], 1))
        com[f"g{g}_Wa1"] = _t2(W @ a[:E])
        com[f"g{g}_WiT"] = _t2(np.ascontiguousarray(Wi.T))
        com[f"g{g}_WhT"] = _t2(np.ascontiguousarray(Wh.T))
        com[f"g{g}_bi"] = np.asarray(gr["bi"], f).reshape(1, -1)
        com[f"g{g}_bh"] = np.asarray(gr["bh"], f).reshape(1, -1)
    for l in range(n_tb):
        tb = params["tblocks"][l]
        gv = lambda k: np.asarray(tb[k], f)
        com[f"t{l}_wq"] = _t2(gv("Wq"))
        com[f"t{l}_wk"] = _t2(gv("Wk"))
        com[f"t{l}_wv"] = _t2(gv("Wv"))
        com[f"t{l}_wo"] = _t2(gv("Wo"))
        com[f"t{l}_bq"] = _t2(gv("bq").reshape(E, 1))
        com[f"t{l}_bk"] = _t2(gv("bk").reshape(E, 1))
        com[f"t{l}_bv"] = gv("bv").reshape(1, E)
        com[f"t{l}_bo"] = gv("bo").reshape(1, E)
        com[f"t{l}_w1"] = _t2(gv("W1"))
        com[f"t{l}_b1"] = _t2(gv("b1").reshape(FFD, 1))
        com[f"t{l}_w2"] = _t2(gv("W2")).astype(ml_dtypes.bfloat16)
        com[f"t{l}_b2"] = gv("b2").reshape(1, E)
        for nm, key in [("ln1g", "ln1_g"), ("ln1b", "ln1_b"),
                        ("ln2g", "ln2_g"), ("ln2b", "ln2_b")]:
            com[f"t{l}_{nm}"] = gv(key).reshape(1, E)

    bf_keys = ["h0T_full"]
    for g in range(n_gg):
        bf_keys += [f"g{g}_{s}" for s in
                    ["Wx", "Wa1", "WiT", "WhT", "bi", "bh"]]
    for l in range(n_tb):
        bf_keys += [f"t{l}_{s}" for s in
                    ["wq", "wk", "wv", "wo", "bv", "bo", "w1", "b2"]]
    for k in bf_keys:
        com[k] = com[k].astype(ml_dtypes.bfloat16)

    in_maps = []
    for r in range(R):
        blk = h0[r * NL:(r + 1) * NL]
        d = dict(com)
        d["h0_loc"] = _t2(blk)
        d["h0_locT"] = _t2(np.ascontiguousarray(blk.T)).astype(
            ml_dtypes.bfloat16)
        d["adjT"] = _t2(np.ascontiguousarray(
            adj[r * NL:(r + 1) * NL].T)).astype(ml_dtypes.bfloat16)
        in_maps.append(d)
    return in_maps, h0


_CACHE = {}


def run_device(in_maps, n_tb=NTB, n_gg=2, trace=False):
    key = (n_tb, n_gg)
    if key not in _CACHE:
        _CACHE[key] = build_bass(n_tb, n_gg)
    nc = _CACHE[key]
    return run_bass_kernel_spmd(nc, in_maps, core_ids=list(range(R)),
                                trace=trace)


def kernel(input_node, inputad, res, inputtext, linenode, modification, churn,
           params, _trace=False, _n_tb=NTB, _n_gg=2):
    in_maps, _ = _prep_inputs(input_node, inputad, res, inputtext, linenode,
                              modification, churn, params, _n_tb, _n_gg)
    out = run_device(in_maps, _n_tb, _n_gg, trace=_trace)
    blocks = [np.asarray(out.results[r]["out_x"]).reshape(NL, E)
              for r in range(R)]
    x_full = np.concatenate(blocks, 0)
    kernel._x_full = x_full
    kernel._profile = out

    f = np.float32
    x = x_full[:NLLEN][None]
    wv = np.asarray(params["res2_w"], f)
    bv = np.asarray(params["res2_b"], f)
    logits = (x @ wv + bv).squeeze(-1).astype(f)
    inode = np.asarray(input_node)
    resmask = inode == 2
    logits = np.where(resmask, logits, f(-1e9)).astype(f)
    zmax = logits.max(-1, keepdims=True)
    ez = np.exp(logits - zmax, dtype=f)
    psm = (ez / ez.sum(-1, keepdims=True)).astype(f)
    loss = np.sum(-np.log(np.clip(psm, 1e-10, 1.0)) * np.asarray(res, f),
                  axis=-1).astype(f)
    return loss, psm, x.astype(f)
